# revision 56
# baseline (speedup 1.0000x reference)
"""Trainium2 Bass kernel for the attention-MLP problem.

Reference computation (S=32768, H=1024):
    cat    = [broadcast(hidden, (S, 2H)) | encoder_output]   # [S, 3H]
    energy = tanh(cat @ attn_w.T + attn_b)                   # [S, H]
    logits = (energy @ v_w.T).squeeze()                      # [S]
    out    = softmax(logits)                                 # [S]

Because the hidden rows are identical, cat @ attn_w.T splits into
    c0  = hidden @ W1T + attn_b          (one row, [H])
    pre = enc @ W2T + c0                  (the real work)
with W1T = attn_w[:, :2H].T and W2T = attn_w[:, 2H:].T.

Sharding: seq axis split across 8 cores (4096 rows each); weights
replicated. Softmax normalization uses exp (no max subtraction needed:
|logits| <= ||v_w||_1 ~ 26, safely inside fp32 exp range) with an
AllGather of the 8 per-core partial sums.

Measured HW facts that shaped the design (marginal For_i benchmarks on
this part):
  * with all 8 cores busy the PE streams a 128x128x512 bf16 matmul in
    ~266-273ns (chip power-state downclock from the 1-core 222ns /
    2.4 GHz rate), so the 512 main matmuls floor at ~136-140us;
    LDWEIGHTS, semaphore updates, and satisfied waits are free in a
    back-to-back stream.
  * same-process decomposition of the previous [j, s] layout: pure
    mains 136us, +10us in-loop enc DMA (mostly bytes-proportional,
    i.e. physics), +1us tanh, +13us vdots/exps -> ~157-160us.

Default schedule (layout="sj", build_sj): energy computed in [s, j]
layout — enc chunks [128k, 128s] are the STATIONARY operand, W2 the
moving one — so the logits contraction over j runs along the free axis
and the entire v-dot disappears from the PE stream (PE does exactly the
512 main matmuls). Per s-tile chain: DVE adds c0 (replicated rows) to
the [128, 1024] psum, ACT tanh -> bf16, DVE multiply by v_rep + reduce
-> one logit per partition; one ACT exp per iteration over the [128,32]
logit tile; cross-partition normalization via a ones-vector matmul +
AllGather. Output is outT [128, 32] (host reassembles s = t*128 + p).
Other details:
  * enc shipped host-pre-tiled ([g, p, k, i, s]) contiguous per
    partition; each tile arrives as two half-DMAs so subtile deps let
    the first s-tiles start on the i=0 half (~3us); 8 tile buffers of
    prefetch depth.
  * LDW dedup post-pass (_dedup_ldweights) drops the second auto-LDW of
    each (t, k) pair (LDWs are free anyway, this just shrinks streams).
  * tensor_tensor_reduce is broken on this walrus ("ISA wrong length"),
    hence the two-pass DVE mul + reduce.
  * bf16 operands (fp8 fails the 2e-2 tolerance: 7.1e-2 measured).
The previous [j, s] layout (energy^T in PSUM + PE v-dots + quadrant
tricks) is kept as build(layout="js") for comparison; it measures
~5-6us slower (156 vs 151us).
"""

import numpy as np

import concourse.bass as bass
import concourse.mybir as mybir
import concourse.tile as tile
from concourse.bass_utils import run_bass_kernel_spmd

H = 1024
S = 32768
NCORES = 8
SL = S // NCORES          # 4096 rows per core
SB = 512                  # seq block (columns of the psum tiles)
NSB = SL // SB            # 8 seq blocks per core
KC = H // 128             # 8 contraction chunks
JC = H // 128             # 8 output-row chunks

F32 = mybir.dt.float32
F32R = mybir.dt.float32r
BF16 = mybir.dt.bfloat16

AF = mybir.ActivationFunctionType


# ---------------------------------------------------------------------------
# Workaround for this walrus build: instructions only accept a single
# sync-wait command, but Tile can attach several. Hoist the extra waits
# onto NOPs inserted just before the instruction on the same engine
# (engines execute their stream in order, so semantics are preserved).
def _split_multi_waits(nc):
    end_bb = nc.cur_bb.bb
    for bb in nc.m.functions[0].blocks:
        insts = list(bb.instructions)
        out = []
        changed = False
        for inst in insts:
            si = inst.sync_info
            waits = list(si.on_wait) if si and si.on_wait else []
            if len(waits) > 1:
                changed = True
                for w in waits[:-1]:
                    nop = nc.engines[inst.engine].nop(nofuse=True).ins
                    end_bb.instructions.remove(nop)
                    nop.sync_info = mybir.SyncInfo(on_wait=[w], on_update=[])
                    out.append(nop)
                si.on_wait = waits[-1:]
            out.append(inst)
        if changed:
            bb.instructions = out
# ---------------------------------------------------------------------------


# Delete LDWEIGHTS that reload weights already resident in the PE array.
# Tile emits one InstLdweights per matmul; when the same stationary
# operand is already loaded at the same array position (and no
# intervening load clobbered its columns), the repeat is pure overhead.
# Position-aware: the array's 32-col strips hold independent weight sets
# (tile_position col tiling), so residency is tracked per column range —
# a new load only clobbers entries whose column ranges intersect.
# Matmuls never clobber loaded weights; fp32/fp32r matmuls self-load
# (clobber all); any other PE instruction conservatively resets tracking.
# Only sync-free LDWs are dropped.
def _dedup_ldweights(nc):
    n_dropped = 0
    for bb in nc.m.functions[0].blocks:
        out = []
        resident = {}   # col_start -> (col_end, key)
        for inst in bb.instructions:
            if inst.engine != mybir.EngineType.PE:
                out.append(inst)
                continue
            if isinstance(inst, mybir.InstLdweights):
                si = inst.sync_info
                has_sync = bool(si and (si.on_wait or si.on_update))
                pos = inst.tile_position or (0, 0)
                size = inst.tile_size or (128, 128)
                c0, c1 = pos[1], pos[1] + size[1]
                key = (
                    str(inst.ins[0]),
                    str(pos),
                    str(size),
                    str(inst.perf_mode),
                    str(inst.is_transpose),
                )
                if resident.get(c0) == (c1, key) and not has_sync:
                    n_dropped += 1
                    continue
                # clobber overlapping column ranges, then install
                resident = {s: (e, k) for s, (e, k) in resident.items()
                            if e <= c0 or s >= c1}
                resident[c0] = (c1, key)
            elif isinstance(inst, mybir.InstMatmult):
                w_dt = inst.ins[1].dtype if len(inst.ins) > 1 else None
                if w_dt in (mybir.dt.float32, mybir.dt.float32r):
                    resident = {}   # self-loading matmul clobbers array
            else:
                resident = {}
            out.append(inst)
        bb.instructions = out
    return n_dropped
# ---------------------------------------------------------------------------


def build_sj(repeat: int = 1, main_dt: str = "bf16",
             single_core: bool = False, enc_bufs: int = 8,
             use_ttr: bool = False, pre_bufs: int = 4, th_bufs: int = 4,
             prod_bufs: int = 3, pre_bf16: bool = False,
             chain: bool = True, dma_split: bool = True):
    """[s, j] energy layout: enc chunks are the stationary operand, W2 the
    moving one, so energy lands as [s-rows, j-cols] in PSUM and the whole
    v-dot disappears from the PE stream — DVE does (psum + c0_rep), ACT
    tanh, then DVE tensor_tensor_reduce(x v_rep, sum) produces one logit
    per partition. PE work: exactly the 512 main matmuls. Output is outT
    [128, SL/128] (host reassembles s = t*128 + p)."""
    MD = {"f32r": F32R, "bf16": BF16}[main_dt]
    NT = SL // 128            # 32 s-tiles per core
    nc = bass.Bass("TRN2", target_bir_lowering=False, debug=False,
                   num_devices=1 if single_core else NCORES)

    encC = nc.dram_tensor("encC", [4, 128, KC * 2 * SB], MD,
                          kind="ExternalInput").ap()
    w2t = nc.dram_tensor("w2t", [H, H], MD, kind="ExternalInput").ap()
    w1t = nc.dram_tensor("w1t", [2 * H // NCORES, H], F32R,
                         kind="ExternalInput").ap()
    hidT = nc.dram_tensor("hidT", [128, 16 // NCORES], F32R,
                          kind="ExternalInput").ap()
    bias = nc.dram_tensor("bias", [1, H], F32, kind="ExternalInput").ap()
    nc.dram_tensor("vwc", [128, JC], BF16, kind="ExternalInput")
    vrep = nc.dram_tensor("vrep", [128, H], BF16,
                          kind="ExternalInput").ap()
    outT = nc.dram_tensor("outT", [128, NT], F32,
                          kind="ExternalOutput").ap()

    encC_v = encC.rearrange("g p (k i s) -> g p k i s", k=KC, i=2)
    w2t_v = w2t.rearrange("(k p) j -> p k j", p=128)     # [128, 8, 1024]
    w1t_v = w1t.rearrange("(k p) j -> p k j", p=128)

    with tile.TileContext(nc) as tc:
        with (
            tc.tile_pool(name="const", bufs=1) as const_pool,
            tc.tile_pool(name="enc", bufs=enc_bufs) as enc_pool,
            tc.tile_pool(name="pre", bufs=pre_bufs) as pre_pool,
            tc.tile_pool(name="tanh", bufs=th_bufs) as tanh_pool,
            tc.tile_pool(name="prod", bufs=prod_bufs) as prod_pool,
            tc.tile_pool(name="sm", bufs=1) as sm_pool,
            tc.tile_pool(name="pse", bufs=4, space="PSUM") as pse_pool,
            tc.tile_pool(name="dram", bufs=1, space="DRAM") as dram_pool,
        ):
            hid_sb = const_pool.tile([128, 16 // NCORES], F32R)
            nc.sync.dma_start(hid_sb[:], hidT[:])
            vrep_sb = const_pool.tile([128, H], BF16)
            nc.sync.dma_start(vrep_sb[:], vrep[:])
            b_sb = const_pool.tile([1, H], F32)
            nc.sync.dma_start(b_sb[:], bias[:])
            w2r = const_pool.tile([128, KC, H], MD)
            nc.sync.dma_start(w2r[:], w2t_v[:])

            logits = sm_pool.tile([128, NT], F32)
            expst = sm_pool.tile([128, NT], F32)
            sumc = sm_pool.tile([128, 1], F32)
            c0_rep = const_pool.tile([128, H], F32)
            # ones row for PE-based partition broadcasts ([1,k]@[1,n] with
            # ones lhsT replicates a row across all 128 output partitions)
            ones_row = const_pool.tile([1, 128], F32)
            nc.gpsimd.memset(ones_row[:], 1.0)

            def bcast_rows(dst_sb, src_row, n):
                # dst_sb [128, n] <- broadcast of src_row [1, n]
                for o in range(0, n, 512):
                    w = min(512, n - o)
                    pb = pse_pool.tile([128, 512], F32, tag="pe",
                                       name="pb")
                    nc.tensor.matmul(pb[:, :w], ones_row[:],
                                     src_row[:, o:o + w],
                                     start=True, stop=True)
                    nc.vector.tensor_copy(dst_sb[:, o:o + w], pb[:, :w])

            NKC = 16 // NCORES

            def c0_section():
                w1_sb = const_pool.tile([128, NKC, H], F32R)
                nc.sync.dma_start(w1_sb[:], w1t_v[:])
                part_row = const_pool.tile([1, H], F32)
                for half in range(2):
                    psum_c = pse_pool.tile([1, 512], F32, tag="pe",
                                           name="psum_c")
                    for kc in range(NKC):
                        nc.tensor.matmul(
                            psum_c[:],
                            hid_sb[:, kc:kc + 1],
                            w1_sb[:, kc, half * 512:(half + 1) * 512],
                            start=(kc == 0), stop=(kc == NKC - 1),
                        )
                    nc.vector.tensor_add(
                        part_row[:, half * 512:(half + 1) * 512],
                        psum_c[:],
                        b_sb[:, half * 512:(half + 1) * 512])
                ar_in = dram_pool.tile([1, H], F32)
                nc.gpsimd.dma_start(ar_in[:], part_row[:])
                if single_core:
                    ar_out = ar_in
                else:
                    ar_out = dram_pool.tile([1, H], F32)
                    nc.gpsimd.collective_compute(
                        "AllReduce",
                        mybir.AluOpType.add,
                        replica_groups=[list(range(NCORES))],
                        ins=[ar_in.opt()],
                        outs=[ar_out.opt()],
                    )
                c0_row = const_pool.tile([1, H], F32)
                nc.sync.dma_start(c0_row[:], ar_out[:])
                bcast_rows(c0_rep, c0_row, H)

            def main_body(_iv=None):
                for h in range(2):
                    enc_ts = []
                    for pp in range(2):
                        enc_t = enc_pool.tile([128, KC, 2, SB], MD,
                                              tag="enc")
                        if dma_split:
                            # two half-DMAs per tile: subtile deps let
                            # the first 4 s-tiles start on the i=0 half
                            for i in range(2):
                                nc.sync.dma_start(
                                    enc_t[:, :, i, :],
                                    encC_v[2 * h + pp][:, :, i, :])
                        else:
                            nc.sync.dma_start(enc_t[:],
                                              encC_v[2 * h + pp])
                        enc_ts.append(enc_t)
                    for tl in range(16):       # s-tiles within the half
                        t = 16 * h + tl
                        q, off = tl // 4, (tl % 4) * 128
                        ps = pse_pool.tile([128, 2, 512], F32, tag="pe",
                                           name="ps")
                        for k in range(KC):
                            st = enc_ts[q // 2][:, k, q % 2,
                                                off:off + 128]
                            for jh in range(2):
                                nc.tensor.matmul(
                                    ps[:, jh, :], st,
                                    w2r[:, k, jh * 512:(jh + 1) * 512],
                                    start=(k == 0), stop=(k == KC - 1),
                                )
                        if not chain:
                            continue
                        pre = pre_pool.tile([128, H],
                                            BF16 if pre_bf16 else F32,
                                            tag="pre", name="pre")
                        nc.vector.tensor_add(
                            pre[:],
                            ps[:].rearrange("p a b -> p (a b)"),
                            c0_rep[:])
                        th = tanh_pool.tile([128, H], BF16, tag="th",
                                            name="th")
                        nc.scalar.activation(th[:], pre[:], AF.Tanh)
                        prod = prod_pool.tile([128, H], BF16, tag="prod",
                                              name="prod")
                        if use_ttr:
                            nc.vector.tensor_tensor_reduce(
                                out=prod[:], in0=th[:], in1=vrep_sb[:],
                                scale=1.0, scalar=0.0,
                                op0=mybir.AluOpType.mult,
                                op1=mybir.AluOpType.add,
                                accum_out=logits[:, t:t + 1])
                        else:
                            nc.vector.tensor_mul(prod[:], th[:],
                                                  vrep_sb[:])
                            nc.vector.tensor_reduce(
                                logits[:, t:t + 1], prod[:],
                                axis=mybir.AxisListType.X,
                                op=mybir.AluOpType.add)
                # per-iteration: exp over all 32 logit columns
                if chain:
                    nc.scalar.activation(expst[:], logits[:], AF.Exp,
                                         accum_out=sumc[:])
                else:
                    nc.gpsimd.memset(expst[:], 1.0)
                    nc.gpsimd.memset(sumc[:], 1.0)

            c0_section()
            if repeat == 1:
                main_body()
            else:
                with tc.For_i(0, repeat, 1,
                              hint_engines=(mybir.EngineType.PE,)) as _i:
                    main_body(_i)

            # --- softmax normalization across cores -----------------------
            ones_sb = sm_pool.tile([128, 1], F32)
            nc.gpsimd.memset(ones_sb[:], 1.0)
            zp = pse_pool.tile([1, 1], F32, tag="pe", name="zp")
            nc.tensor.matmul(zp[:], ones_sb[:], sumc[:],
                             start=True, stop=True)
            if single_core:
                zg_src = zp
            else:
                ag_in = dram_pool.tile([1, 1], F32)
                zsb = sm_pool.tile([1, 1], F32)
                nc.vector.tensor_copy(zsb[:], zp[:])
                nc.gpsimd.dma_start(ag_in[:], zsb[:])
                ag_out = dram_pool.tile([1, NCORES], F32)
                nc.gpsimd.collective_compute(
                    "AllGather",
                    mybir.AluOpType.bypass,
                    replica_groups=[list(range(NCORES))],
                    ins=[ag_in.opt()],
                    outs=[ag_out.opt()],
                )
                zs = sm_pool.tile([1, NCORES], F32)
                nc.gpsimd.dma_start(zs[:], ag_out[:])
                zg_src = None
            zg = sm_pool.tile([1, 1], F32)
            if single_core:
                nc.vector.tensor_copy(zg[:], zg_src[:])
            else:
                nc.vector.reduce_sum(zg[:], zs[:], axis=mybir.AxisListType.X)
            invz = sm_pool.tile([1, 1], F32)
            nc.vector.reciprocal(invz[:], zg[:])
            invz_rep = sm_pool.tile([128, 1], F32)
            bcast_rows(invz_rep, invz, 1)
            outv = sm_pool.tile([128, NT], F32)
            nc.vector.tensor_scalar_mul(outv[:], expst[:], invz_rep[:])
            nc.sync.dma_start(outT[:], outv[:])

    _dedup_ldweights(nc)
    _split_multi_waits(nc)
    return nc


def build(repeat: int = 1, main_dt: str = "bf16", single_core: bool = False,
          mode: str = "full", exp_sbuf: bool = True, enc_bufs: int = 8,
          tanh_bufs: int = 10, vdot_preload: bool = True,
          vdot_batch: bool = True, dma_rings: bool = False,
          dma_fuse: bool = False, layout: str = "sj", **sj_kw):
    if layout == "sj":
        return build_sj(repeat, main_dt=main_dt, single_core=single_core,
                        enc_bufs=enc_bufs, **sj_kw)
    """Build the per-core Bass module. `repeat` wraps the main compute in a
    For_i loop (used only by the benchmark harness to measure HW time by
    marginal wall-clock; the softmax tail + collective stay outside).
    mode: full | mm_only (perf experiment: main matmuls + dma only) |
    mm_tanh (mains + dma + tanh, no vdots/exps) | mm_resident (main
    matmuls only, enc preloaded to SBUF outside the loop)."""
    mm_only = mode in ("mm_only", "mm_resident", "mm_halfdma")
    mm_resident = mode == "mm_resident"
    half_dma = mode == "mm_halfdma"
    do_tanh = mode in ("full", "mm_tanh")
    do_vdot = mode == "full"
    if vdot_batch:
        # a full half's th tiles (32) stay alive until the burst, plus
        # the next half's first groups in flight
        tanh_bufs = max(tanh_bufs, 38)
    MD = {"f32r": F32R, "bf16": BF16}[main_dt]
    nc = bass.Bass("TRN2", target_bir_lowering=False, debug=False,
                   num_devices=1 if single_core else NCORES)

    # enc shard pre-tiled on host: [g, p, (k i s)] with g = 1KB s-block
    # group, so every per-tile DMA reads 16KB contiguous per partition
    # (128 descriptors instead of 1024 for the strided [H, SL] layout).
    encC = nc.dram_tensor("encC", [4, 128, KC * 2 * SB], MD,
                          kind="ExternalInput").ap()
    w2t = nc.dram_tensor("w2t", [H, H], MD, kind="ExternalInput").ap()
    w1t = nc.dram_tensor("w1t", [2 * H // NCORES, H], F32R,
                         kind="ExternalInput").ap()
    hidT = nc.dram_tensor("hidT", [128, 16 // NCORES], F32R,
                          kind="ExternalInput").ap()
    bias = nc.dram_tensor("bias", [1, H], F32, kind="ExternalInput").ap()
    vwc = nc.dram_tensor("vwc", [128, JC], BF16, kind="ExternalInput").ap()
    # declared by both layouts so one in_map serves either build
    nc.dram_tensor("vrep", [128, H], BF16, kind="ExternalInput")
    out = nc.dram_tensor("out", [1, SL], F32, kind="ExternalOutput").ap()

    # [4, 128, 8, 2, 512]: g-th 1KB s-block group, contiguous per partition
    encC_v = encC.rearrange("g p (k i s) -> g p k i s", k=KC, i=2)
    w2t_v = w2t.rearrange("(k p) j -> p k j", p=128)     # [128, 8, 1024]
    w1t_v = w1t.rearrange("(k p) j -> p k j", p=128)     # [128, 2, 1024]

    with tile.TileContext(nc) as tc:
        with (
            tc.tile_pool(name="const", bufs=1) as const_pool,
            tc.tile_pool(name="enc", bufs=enc_bufs) as enc_pool,
            tc.tile_pool(name="tanh", bufs=tanh_bufs) as tanh_pool,
            tc.tile_pool(name="sm", bufs=1) as sm_pool,
            tc.tile_pool(name="pse", bufs=7, space="PSUM") as pse_pool,
            tc.tile_pool(name="psa", bufs=1, space="PSUM") as psa_pool,
            tc.tile_pool(name="dram", bufs=1, space="DRAM") as dram_pool,
        ):
            # --- tiny constants -------------------------------------------
            hid_sb = const_pool.tile([128, 16 // NCORES], F32R)
            nc.sync.dma_start(hid_sb[:], hidT[:])
            vw_sb = const_pool.tile([128, JC], BF16)
            nc.sync.dma_start(vw_sb[:], vwc[:])
            b_sb = const_pool.tile([1, H], F32)
            nc.sync.dma_start(b_sb[:], bias[:])

            # --- replicated weights: one tile per j-slab so the group-j
            # matmuls depend only on their own slab's DMA ---------------
            w2_tiles = []
            for j in range(JC):
                w2_j = const_pool.tile([128, KC, 128], MD, name=f"w2_{j}")
                nc.sync.dma_start(w2_j[:], w2t_v[:, :, j * 128:(j + 1) * 128])
                w2_tiles.append(w2_j)

            exps = sm_pool.tile([1, SL], F32)
            sums = sm_pool.tile([1, NSB], F32)

            # --- c0 = hidden @ W1T + attn_b (one row), sharded over cores
            c0_sb = const_pool.tile([128, JC], F32)

            NKC = 16 // NCORES   # local w1 chunks (c0 sharded over cores)

            def c0_section():
                w1_sb = const_pool.tile([128, NKC, H], F32R)
                nc.sync.dma_start(w1_sb[:], w1t_v[:])
                # bias arrives pre-divided by NCORES, so adding it to the
                # local partial and AllReduce-summing reconstructs c0+b
                part_row = const_pool.tile([1, H], F32)
                for half in range(2):
                    psum_c = pse_pool.tile([1, 512], F32, tag="pe",
                                           name="psum_c")
                    for kc in range(NKC):
                        nc.tensor.matmul(
                            psum_c[:],
                            hid_sb[:, kc:kc + 1],
                            w1_sb[:, kc, half * 512:(half + 1) * 512],
                            start=(kc == 0), stop=(kc == NKC - 1),
                        )
                    nc.vector.tensor_add(
                        part_row[:, half * 512:(half + 1) * 512],
                        psum_c[:],
                        b_sb[:, half * 512:(half + 1) * 512])
                ar_in = dram_pool.tile([1, H], F32)
                nc.gpsimd.dma_start(ar_in[:], part_row[:])
                if single_core:
                    ar_out = ar_in
                else:
                    ar_out = dram_pool.tile([1, H], F32)
                    nc.gpsimd.collective_compute(
                        "AllReduce",
                        mybir.AluOpType.add,
                        replica_groups=[list(range(NCORES))],
                        ins=[ar_in.opt()],
                        outs=[ar_out.opt()],
                    )
                nc.sync.dma_start(
                    c0_sb[:],
                    ar_out[:].rearrange("o (j p) -> (o p) j", p=128)
                )

            # --- main pipeline -------------------------------------------
            enc_res = [None]
            if mm_resident:
                enc_res[0] = const_pool.tile([128, 4, KC, 2, SB], MD,
                                             name="enc_res")
                for g in range(4):
                    nc.sync.dma_start(enc_res[0][:, g], encC_v[g])

            def main_body(_iv=None):
                # per j-group: 4 single-bank psum accumulators (the 4
                # s-blocks of the half), all fed k-outer so the 4 matmuls
                # of a (j, k) pair share one weight load. One [128, SB]
                # psum_a bank whose quadrant rows 0/32/64/96 hold the 4
                # s-blocks' logits so the 4 v-dots of a group land on
                # distinct PE column groups and stream concurrently.
                psum_a = [None]
                pending = []               # delayed v-dot emissions
                last_main = [None]         # latest main matmul instruction

                def flush():
                    for emit in pending:
                        emit()
                    pending.clear()

                def make_vdot(j, ths, pa):
                    def emit():
                        if vdot_preload:
                            # preload all 4 col-group weight slots first,
                            # then issue the 4 matmuls back-to-back so
                            # they stream concurrently (no interleaved
                            # LDW can stall the col-group pipeline; the
                            # per-MM auto-LDWs dedup against these).
                            # Pin each preload behind the latest main
                            # matmul so the scheduler cannot hoist it
                            # into an earlier weight-load's live range.
                            for q in range(4):
                                ldw = nc.tensor.ldweights(
                                    vw_sb[:, j:j + 1],
                                    tile_position=(0, 32 * q))
                                # mirror the fused matmul's rounded tile
                                # size so the per-MM auto-LDW dedups
                                # against this preload
                                ldw.ins.tile_size = (128, 32)
                                if last_main[0] is not None:
                                    bass._add_dep_helper(
                                        ldw.ins, last_main[0],
                                        sync=True,
                                        reason="pin vdot preload")
                        for q in range(4):
                            r = 32 * q
                            nc.tensor.matmul(
                                pa[r:r + 1, :],
                                vw_sb[:, j:j + 1], ths[q][:],
                                tile_position=(0, r),
                                start=(j == 0), stop=(j == JC - 1),
                            )
                    return emit

                def copy_logits(pa):
                    # DVE copies the logits bank to SBUF (~0.7us) so the
                    # psa bank frees fast and ACT's exps read SBUF off the
                    # PE-critical path (DVE is otherwise idle in-loop)
                    if not exp_sbuf:
                        return pa
                    lt = sm_pool.tile([128, SB], F32, tag="lt", name="lt",
                                      bufs=2)
                    nc.vector.tensor_copy(lt[:], pa[:])
                    return lt

                def emit_exps(h, lt):
                    for q in range(4):
                        sb = 4 * h + q
                        nc.scalar.activation(
                            exps[:, sb * SB:(sb + 1) * SB],
                            lt[32 * q:32 * q + 1, :], AF.Exp,
                            accum_out=sums[:, sb:sb + 1],
                        )

                prev_pa = None
                for h in range(2):
                    if mm_resident:
                        enc_ts = [enc_res[0][:, 2 * h + pp]
                                  for pp in range(2)]
                    elif dma_fuse:
                        # one 4MB DMA per half covering both s-block pairs
                        enc_t2 = enc_pool.tile([128, 2, KC, 2, SB], MD,
                                               tag="enc", bufs=enc_bufs // 2)
                        nc.sync.dma_start(
                            enc_t2[:],
                            encC.rearrange("g p x -> p g x")[:, 2 * h:2 * h + 2]
                            .rearrange("p g (k i s) -> p g k i s", k=KC, i=2),
                        )
                        enc_ts = [enc_t2[:, pp] for pp in range(2)]
                    else:
                        enc_ts = []
                        for pp in range(2):     # two s-block pairs per half
                            enc_t = enc_pool.tile([128, KC, 2, SB], MD,
                                                  tag="enc")
                            eng = (nc.scalar if (dma_rings and pp == 1)
                                   else nc.sync)
                            if half_dma:   # perf probe: half the bytes
                                eng.dma_start(enc_t[:, :KC // 2],
                                              encC_v[2 * h + pp][:, :KC // 2])
                            else:
                                eng.dma_start(enc_t[:], encC_v[2 * h + pp])
                            enc_ts.append(enc_t)
                    for j in range(JC):
                        pes = [
                            pse_pool.tile([128, SB], F32, tag="pe",
                                          name="pe")
                            for _ in range(4)
                        ]
                        for k in range(KC):
                            w = w2_tiles[j][:, k, :]
                            for q in range(4):
                                mm = nc.tensor.matmul(
                                    pes[q][:], w,
                                    enc_ts[q // 2][:, k, q % 2, :],
                                    start=(k == 0), stop=(k == KC - 1),
                                )
                                last_main[0] = mm.ins
                        if not do_tanh:
                            continue
                        if not vdot_batch or j == 0:
                            flush()
                        lt_prev = None
                        if do_vdot and j == 0:
                            # previous half's logits complete: DVE-copy
                            # them out before this half's first v-dots
                            # reuse the bank
                            if h == 1:
                                lt_prev = copy_logits(prev_pa)
                            psum_a[0] = psa_pool.tile(
                                [128, SB], F32, tag="psa", name="psa")
                        ths = []
                        for q in range(4):
                            th = tanh_pool.tile([128, SB], BF16,
                                                tag="th", name="th")
                            nc.scalar.activation(
                                th[:], pes[q][:], AF.Tanh,
                                bias=c0_sb[:, j:j + 1])
                            ths.append(th)
                        if lt_prev is not None:
                            # exps queue on ACT after this j's tanhs so
                            # they never delay the psum-bank recycle
                            emit_exps(0, lt_prev)
                        if do_vdot:
                            pending.append(make_vdot(j, ths, psum_a[0]))
                    prev_pa = psum_a[0]
                if do_vdot:
                    flush()
                    emit_exps(1, copy_logits(prev_pa))
                else:
                    nc.gpsimd.memset(exps[:], 1.0)
                    nc.gpsimd.memset(sums[:], 1.0)

            c0_section()
            if repeat == 1:
                main_body()
            else:
                with tc.For_i(0, repeat, 1,
                              hint_engines=(mybir.EngineType.PE,)) as _i:
                    main_body(_i)

            # --- softmax normalization across cores -----------------------
            if single_core:
                zg = sm_pool.tile([1, 1], F32)
                nc.vector.reduce_sum(zg[:], sums[:],
                                     axis=mybir.AxisListType.X)
            else:
                # AllGather the raw per-block sums (8 floats/core) and do a
                # single 64-element reduce afterwards
                ag_in = dram_pool.tile([1, NSB], F32)
                nc.gpsimd.dma_start(ag_in[:], sums[:])
                ag_out = dram_pool.tile([1, NCORES * NSB], F32)
                nc.gpsimd.collective_compute(
                    "AllGather",
                    mybir.AluOpType.bypass,
                    replica_groups=[list(range(NCORES))],
                    ins=[ag_in.opt()],
                    outs=[ag_out.opt()],
                )
                zs = sm_pool.tile([1, NCORES * NSB], F32)
                nc.gpsimd.dma_start(zs[:], ag_out[:])
                zg = sm_pool.tile([1, 1], F32)
                nc.vector.reduce_sum(zg[:], zs[:], axis=mybir.AxisListType.X)
            invz = sm_pool.tile([1, 1], F32)
            nc.vector.reciprocal(invz[:], zg[:])
            outv = sm_pool.tile([1, SL], F32)
            # split the 4096-element scale across ACT and DVE in parallel,
            # and ship each half as soon as it's done
            hl = SL // 2
            nc.scalar.activation(outv[:, :hl], exps[:, :hl], AF.Identity,
                                 scale=invz[:])
            nc.sync.dma_start(out[:, :hl], outv[:, :hl])
            nc.vector.tensor_scalar_mul(outv[:, hl:], exps[:, hl:], invz[:])
            nc.sync.dma_start(out[:, hl:], outv[:, hl:])

    _dedup_ldweights(nc)
    _split_multi_waits(nc)
    return nc


def prepare_in_maps(hidden, encoder_output, attn_w, attn_b, v_w,
                    main_dt="bf16"):
    hidden = np.asarray(hidden, dtype=np.float32)
    enc = np.asarray(encoder_output, dtype=np.float32)
    attn_w = np.asarray(attn_w, dtype=np.float32)
    attn_b = np.asarray(attn_b, dtype=np.float32)
    v_w = np.asarray(v_w, dtype=np.float32)

    import ml_dtypes
    md = np.float32 if main_dt == "f32r" else ml_dtypes.bfloat16
    w2t = np.ascontiguousarray(attn_w[:, 2 * H:].T).astype(md)   # [H, H]
    w1t_full = np.ascontiguousarray(attn_w[:, :2 * H].T)
    hidT_full = np.ascontiguousarray(hidden.reshape(16, 128).T)
    kpc = 16 // NCORES
    b = np.ascontiguousarray(attn_b.reshape(1, H)) / np.float32(NCORES)
    vwc = np.ascontiguousarray(v_w.reshape(JC, 128).T).astype(
        ml_dtypes.bfloat16)  # [128, 8]
    vrep = np.ascontiguousarray(
        np.broadcast_to(v_w.reshape(1, H), (128, H))).astype(
        ml_dtypes.bfloat16)  # [128, H] replicated

    in_maps = []
    for c in range(NCORES):
        encT = enc[c * SL:(c + 1) * SL, :].T.astype(md)   # [H, SL]
        # [g, p, k, i, s]: per-partition-contiguous tile layout so each
        # 2MB tile DMA needs only 128 descriptors of 16KB
        encC = np.ascontiguousarray(
            encT.reshape(KC, 128, 4, 2, SB).transpose(2, 1, 0, 3, 4)
        ).reshape(4, 128, KC * 2 * SB)
        in_maps.append({
            "encC": encC, "w2t": w2t,
            "w1t": np.ascontiguousarray(
                w1t_full[c * kpc * 128:(c + 1) * kpc * 128, :]),
            "hidT": np.ascontiguousarray(
                hidT_full[:, c * kpc:(c + 1) * kpc]),
            "bias": b, "vwc": vwc, "vrep": vrep,
        })
    return in_maps


_NC_CACHE = {}


def _get_nc(repeat: int = 1):
    if repeat not in _NC_CACHE:
        _NC_CACHE[repeat] = build(repeat)
    return _NC_CACHE[repeat]


def kernel(hidden, encoder_output, attn_w, attn_b, v_w):
    nc = _get_nc(1)
    in_maps = prepare_in_maps(hidden, encoder_output, attn_w, attn_b, v_w)
    res = run_bass_kernel_spmd(nc, in_maps, list(range(NCORES)))
    parts = []
    for c in range(NCORES):
        r = res.results[c]
        if "outT" in r:
            # outT[p, t] holds s_local = t*128 + p
            parts.append(np.ascontiguousarray(r["outT"].T).reshape(SL))
        else:
            parts.append(r["out"][0])
    return np.concatenate(parts)


# revision 58
# speedup vs baseline: 1.0195x; 1.0195x over previous
"""Trainium2 Bass kernel for the attention-MLP problem.

Reference computation (S=32768, H=1024):
    cat    = [broadcast(hidden, (S, 2H)) | encoder_output]   # [S, 3H]
    energy = tanh(cat @ attn_w.T + attn_b)                   # [S, H]
    logits = (energy @ v_w.T).squeeze()                      # [S]
    out    = softmax(logits)                                 # [S]

Because the hidden rows are identical, cat @ attn_w.T splits into
    c0  = hidden @ W1T + attn_b          (one row, [H])
    pre = enc @ W2T + c0                  (the real work)
with W1T = attn_w[:, :2H].T and W2T = attn_w[:, 2H:].T.

Sharding: seq axis split across 8 cores (4096 rows each); weights
replicated. Softmax normalization uses exp (no max subtraction needed:
|logits| <= ||v_w||_1 ~ 26, safely inside fp32 exp range) with an
AllGather of the 8 per-core partial sums.

Measured HW facts that shaped the design (marginal For_i benchmarks on
this part):
  * with all 8 cores busy the PE streams a 128x128x512 bf16 matmul in
    ~266-273ns (chip power-state downclock from the 1-core 222ns /
    2.4 GHz rate), so the 512 main matmuls floor at ~136-140us;
    LDWEIGHTS, semaphore updates, and satisfied waits are free in a
    back-to-back stream.
  * same-process decomposition of the previous [j, s] layout: pure
    mains 136us, +10us in-loop enc DMA (mostly bytes-proportional,
    i.e. physics), +1us tanh, +13us vdots/exps -> ~157-160us.

Default schedule (layout="sj", build_sj): energy computed in [s, j]
layout — enc chunks [128k, 128s] are the STATIONARY operand, W2 the
moving one — so the logits contraction over j runs along the free axis
and the entire v-dot disappears from the PE stream (PE does exactly the
512 main matmuls). Per s-tile chain: DVE adds c0 (replicated rows) to
the [128, 1024] psum, ACT tanh -> bf16, DVE multiply by v_rep + reduce
-> one logit per partition; one ACT exp per iteration over the [128,32]
logit tile; cross-partition normalization via a ones-vector matmul +
AllGather. Output is outT [128, 32] (host reassembles s = t*128 + p).
Other details:
  * enc shipped host-pre-tiled ([g, p, k, i, s]) contiguous per
    partition; each tile arrives as two half-DMAs so subtile deps let
    the first s-tiles start on the i=0 half (~3us); 8 tile buffers of
    prefetch depth.
  * LDW dedup post-pass (_dedup_ldweights) drops the second auto-LDW of
    each (t, k) pair (LDWs are free anyway, this just shrinks streams).
  * tensor_tensor_reduce is broken on this walrus ("ISA wrong length"),
    hence the two-pass DVE mul + reduce.
  * bf16 operands (fp8 fails the 2e-2 tolerance: 7.1e-2 measured).
The previous [j, s] layout (energy^T in PSUM + PE v-dots + quadrant
tricks) is kept as build(layout="js") for comparison; it measures
~5-6us slower (156 vs 151us).
"""

import numpy as np

import concourse.bass as bass
import concourse.mybir as mybir
import concourse.tile as tile
from concourse.bass_utils import run_bass_kernel_spmd

H = 1024
S = 32768
NCORES = 8
SL = S // NCORES          # 4096 rows per core
SB = 512                  # seq block (columns of the psum tiles)
NSB = SL // SB            # 8 seq blocks per core
KC = H // 128             # 8 contraction chunks
JC = H // 128             # 8 output-row chunks

F32 = mybir.dt.float32
F32R = mybir.dt.float32r
BF16 = mybir.dt.bfloat16

AF = mybir.ActivationFunctionType


# ---------------------------------------------------------------------------
# Workaround for this walrus build: instructions only accept a single
# sync-wait command, but Tile can attach several. Hoist the extra waits
# onto NOPs inserted just before the instruction on the same engine
# (engines execute their stream in order, so semantics are preserved).
def _split_multi_waits(nc):
    end_bb = nc.cur_bb.bb
    for bb in nc.m.functions[0].blocks:
        insts = list(bb.instructions)
        out = []
        changed = False
        for inst in insts:
            si = inst.sync_info
            waits = list(si.on_wait) if si and si.on_wait else []
            if len(waits) > 1:
                changed = True
                for w in waits[:-1]:
                    nop = nc.engines[inst.engine].nop(nofuse=True).ins
                    end_bb.instructions.remove(nop)
                    nop.sync_info = mybir.SyncInfo(on_wait=[w], on_update=[])
                    out.append(nop)
                si.on_wait = waits[-1:]
            out.append(inst)
        if changed:
            bb.instructions = out
# ---------------------------------------------------------------------------


# Delete LDWEIGHTS that reload weights already resident in the PE array.
# Tile emits one InstLdweights per matmul; when the same stationary
# operand is already loaded at the same array position (and no
# intervening load clobbered its columns), the repeat is pure overhead.
# Position-aware: the array's 32-col strips hold independent weight sets
# (tile_position col tiling), so residency is tracked per column range —
# a new load only clobbers entries whose column ranges intersect.
# Matmuls never clobber loaded weights; fp32/fp32r matmuls self-load
# (clobber all); any other PE instruction conservatively resets tracking.
# Only sync-free LDWs are dropped.
def _dedup_ldweights(nc):
    n_dropped = 0
    for bb in nc.m.functions[0].blocks:
        out = []
        resident = {}   # col_start -> (col_end, key)
        for inst in bb.instructions:
            if inst.engine != mybir.EngineType.PE:
                out.append(inst)
                continue
            if isinstance(inst, mybir.InstLdweights):
                si = inst.sync_info
                has_sync = bool(si and (si.on_wait or si.on_update))
                pos = inst.tile_position or (0, 0)
                size = inst.tile_size or (128, 128)
                c0, c1 = pos[1], pos[1] + size[1]
                key = (
                    str(inst.ins[0]),
                    str(pos),
                    str(size),
                    str(inst.perf_mode),
                    str(inst.is_transpose),
                )
                if resident.get(c0) == (c1, key) and not has_sync:
                    n_dropped += 1
                    continue
                # clobber overlapping column ranges, then install
                resident = {s: (e, k) for s, (e, k) in resident.items()
                            if e <= c0 or s >= c1}
                resident[c0] = (c1, key)
            elif isinstance(inst, mybir.InstMatmult):
                w_dt = inst.ins[1].dtype if len(inst.ins) > 1 else None
                if w_dt in (mybir.dt.float32, mybir.dt.float32r):
                    resident = {}   # self-loading matmul clobbers array
            else:
                resident = {}
            out.append(inst)
        bb.instructions = out
    return n_dropped
# ---------------------------------------------------------------------------


def build_sj(repeat: int = 1, main_dt: str = "bf16",
             single_core: bool = False, enc_bufs: int = 8,
             use_ttr: bool = False, pre_bufs: int = 4, th_bufs: int = 4,
             prod_bufs: int = 3, pre_bf16: bool = False,
             chain: bool = True, dma_split: bool = True):
    """[s, j] energy layout: enc chunks are the stationary operand, W2 the
    moving one, so energy lands as [s-rows, j-cols] in PSUM and the whole
    v-dot disappears from the PE stream — DVE does (psum + c0_rep), ACT
    tanh, then DVE tensor_tensor_reduce(x v_rep, sum) produces one logit
    per partition. PE work: exactly the 512 main matmuls. Output is outT
    [128, SL/128] (host reassembles s = t*128 + p)."""
    MD = {"f32r": F32R, "bf16": BF16}[main_dt]
    NT = SL // 128            # 32 s-tiles per core
    nc = bass.Bass("TRN2", target_bir_lowering=False, debug=False,
                   num_devices=1 if single_core else NCORES)

    encC = nc.dram_tensor("encC", [4, 2, 128, KC * SB], MD,
                          kind="ExternalInput").ap()
    w2t = nc.dram_tensor("w2t", [H, H], MD, kind="ExternalInput").ap()
    w1t = nc.dram_tensor("w1t", [2 * H // NCORES, H], F32R,
                         kind="ExternalInput").ap()
    hidT = nc.dram_tensor("hidT", [128, 16 // NCORES], F32R,
                          kind="ExternalInput").ap()
    bias = nc.dram_tensor("bias", [1, H], F32, kind="ExternalInput").ap()
    nc.dram_tensor("vwc", [128, JC], BF16, kind="ExternalInput")
    vrep = nc.dram_tensor("vrep", [128, H], BF16,
                          kind="ExternalInput").ap()
    outT = nc.dram_tensor("outT", [128, NT], F32,
                          kind="ExternalOutput").ap()

    # [g, i, p, k, s]: each (g, i) half is contiguous per partition, so a
    # half-DMA needs only 128 descriptors of 8KB
    encC_v = encC.rearrange("g i p (k s) -> g i p k s", k=KC)
    w2t_v = w2t.rearrange("(k p) j -> p k j", p=128)     # [128, 8, 1024]
    w1t_v = w1t.rearrange("(k p) j -> p k j", p=128)

    with tile.TileContext(nc) as tc:
        with (
            tc.tile_pool(name="const", bufs=1) as const_pool,
            tc.tile_pool(name="enc", bufs=enc_bufs) as enc_pool,
            tc.tile_pool(name="pre", bufs=pre_bufs) as pre_pool,
            tc.tile_pool(name="tanh", bufs=th_bufs) as tanh_pool,
            tc.tile_pool(name="prod", bufs=prod_bufs) as prod_pool,
            tc.tile_pool(name="sm", bufs=1) as sm_pool,
            tc.tile_pool(name="pse", bufs=4, space="PSUM") as pse_pool,
            tc.tile_pool(name="dram", bufs=1, space="DRAM") as dram_pool,
        ):
            hid_sb = const_pool.tile([128, 16 // NCORES], F32R)
            nc.sync.dma_start(hid_sb[:], hidT[:])
            vrep_sb = const_pool.tile([128, H], BF16)
            nc.sync.dma_start(vrep_sb[:], vrep[:])
            b_sb = const_pool.tile([1, H], F32)
            nc.sync.dma_start(b_sb[:], bias[:])
            w2r = const_pool.tile([128, KC, H], MD)
            nc.sync.dma_start(w2r[:], w2t_v[:])

            logits = sm_pool.tile([128, NT], F32)
            expst = sm_pool.tile([128, NT], F32)
            sumc = sm_pool.tile([128, 1], F32)
            c0_rep = const_pool.tile([128, H], F32)
            # ones row for PE-based partition broadcasts ([1,k]@[1,n] with
            # ones lhsT replicates a row across all 128 output partitions)
            ones_row = const_pool.tile([1, 128], F32)
            nc.gpsimd.memset(ones_row[:], 1.0)

            def bcast_rows(dst_sb, src_row, n):
                # dst_sb [128, n] <- broadcast of src_row [1, n]
                for o in range(0, n, 512):
                    w = min(512, n - o)
                    pb = pse_pool.tile([128, 512], F32, tag="pe",
                                       name="pb")
                    nc.tensor.matmul(pb[:, :w], ones_row[:],
                                     src_row[:, o:o + w],
                                     start=True, stop=True)
                    nc.vector.tensor_copy(dst_sb[:, o:o + w], pb[:, :w])

            NKC = 16 // NCORES

            def c0_section():
                w1_sb = const_pool.tile([128, NKC, H], F32R)
                nc.sync.dma_start(w1_sb[:], w1t_v[:])
                part_row = const_pool.tile([1, H], F32)
                for half in range(2):
                    psum_c = pse_pool.tile([1, 512], F32, tag="pe",
                                           name="psum_c")
                    for kc in range(NKC):
                        nc.tensor.matmul(
                            psum_c[:],
                            hid_sb[:, kc:kc + 1],
                            w1_sb[:, kc, half * 512:(half + 1) * 512],
                            start=(kc == 0), stop=(kc == NKC - 1),
                        )
                    nc.vector.tensor_add(
                        part_row[:, half * 512:(half + 1) * 512],
                        psum_c[:],
                        b_sb[:, half * 512:(half + 1) * 512])
                ar_in = dram_pool.tile([1, H], F32)
                nc.gpsimd.dma_start(ar_in[:], part_row[:])
                if single_core:
                    ar_out = ar_in
                else:
                    ar_out = dram_pool.tile([1, H], F32)
                    nc.gpsimd.collective_compute(
                        "AllReduce",
                        mybir.AluOpType.add,
                        replica_groups=[list(range(NCORES))],
                        ins=[ar_in.opt()],
                        outs=[ar_out.opt()],
                    )
                c0_row = const_pool.tile([1, H], F32)
                nc.sync.dma_start(c0_row[:], ar_out[:])
                bcast_rows(c0_rep, c0_row, H)

            def main_body(_iv=None):
                for h in range(2):
                    enc_ts = []
                    for pp in range(2):
                        enc_t = enc_pool.tile([128, 2, KC, SB], MD,
                                              tag="enc")
                        if dma_split:
                            # two contiguous half-DMAs per tile: subtile
                            # deps let the first 4 s-tiles start on the
                            # i=0 half
                            for i in range(2):
                                nc.sync.dma_start(
                                    enc_t[:, i],
                                    encC_v[2 * h + pp, i])
                        else:
                            nc.sync.dma_start(
                                enc_t[:],
                                encC_v[2 * h + pp]
                                .rearrange("i p k s -> p i k s"))
                        enc_ts.append(enc_t)
                    for tl in range(16):       # s-tiles within the half
                        t = 16 * h + tl
                        q, off = tl // 4, (tl % 4) * 128
                        ps = pse_pool.tile([128, 2, 512], F32, tag="pe",
                                           name="ps")
                        for k in range(KC):
                            st = enc_ts[q // 2][:, q % 2, k,
                                                off:off + 128]
                            for jh in range(2):
                                nc.tensor.matmul(
                                    ps[:, jh, :], st,
                                    w2r[:, k, jh * 512:(jh + 1) * 512],
                                    start=(k == 0), stop=(k == KC - 1),
                                )
                        if not chain:
                            continue
                        pre = pre_pool.tile([128, H],
                                            BF16 if pre_bf16 else F32,
                                            tag="pre", name="pre")
                        nc.vector.tensor_add(
                            pre[:],
                            ps[:].rearrange("p a b -> p (a b)"),
                            c0_rep[:])
                        th = tanh_pool.tile([128, H], BF16, tag="th",
                                            name="th")
                        nc.scalar.activation(th[:], pre[:], AF.Tanh)
                        prod = prod_pool.tile([128, H], BF16, tag="prod",
                                              name="prod")
                        if use_ttr:
                            nc.vector.tensor_tensor_reduce(
                                out=prod[:], in0=th[:], in1=vrep_sb[:],
                                scale=1.0, scalar=0.0,
                                op0=mybir.AluOpType.mult,
                                op1=mybir.AluOpType.add,
                                accum_out=logits[:, t:t + 1])
                        else:
                            nc.vector.tensor_mul(prod[:], th[:],
                                                  vrep_sb[:])
                            nc.vector.tensor_reduce(
                                logits[:, t:t + 1], prod[:],
                                axis=mybir.AxisListType.X,
                                op=mybir.AluOpType.add)
                # per-iteration: exp over all 32 logit columns
                if chain:
                    nc.scalar.activation(expst[:], logits[:], AF.Exp,
                                         accum_out=sumc[:])
                else:
                    nc.gpsimd.memset(expst[:], 1.0)
                    nc.gpsimd.memset(sumc[:], 1.0)

            c0_section()
            if repeat == 1:
                main_body()
            else:
                with tc.For_i(0, repeat, 1,
                              hint_engines=(mybir.EngineType.PE,)) as _i:
                    main_body(_i)

            # --- softmax normalization across cores -----------------------
            ones_sb = sm_pool.tile([128, 1], F32)
            nc.gpsimd.memset(ones_sb[:], 1.0)
            zp = pse_pool.tile([1, 1], F32, tag="pe", name="zp")
            nc.tensor.matmul(zp[:], ones_sb[:], sumc[:],
                             start=True, stop=True)
            if single_core:
                zg_src = zp
            else:
                ag_in = dram_pool.tile([1, 1], F32)
                zsb = sm_pool.tile([1, 1], F32)
                nc.vector.tensor_copy(zsb[:], zp[:])
                nc.gpsimd.dma_start(ag_in[:], zsb[:])
                ag_out = dram_pool.tile([1, NCORES], F32)
                nc.gpsimd.collective_compute(
                    "AllGather",
                    mybir.AluOpType.bypass,
                    replica_groups=[list(range(NCORES))],
                    ins=[ag_in.opt()],
                    outs=[ag_out.opt()],
                )
                zs = sm_pool.tile([1, NCORES], F32)
                nc.gpsimd.dma_start(zs[:], ag_out[:])
                zg_src = None
            zg = sm_pool.tile([1, 1], F32)
            if single_core:
                nc.vector.tensor_copy(zg[:], zg_src[:])
            else:
                nc.vector.reduce_sum(zg[:], zs[:], axis=mybir.AxisListType.X)
            invz = sm_pool.tile([1, 1], F32)
            nc.vector.reciprocal(invz[:], zg[:])
            invz_rep = sm_pool.tile([128, 1], F32)
            bcast_rows(invz_rep, invz, 1)
            outv = sm_pool.tile([128, NT], F32)
            nc.vector.tensor_scalar_mul(outv[:], expst[:], invz_rep[:])
            nc.sync.dma_start(outT[:], outv[:])

    _dedup_ldweights(nc)
    _split_multi_waits(nc)
    return nc


def build(repeat: int = 1, main_dt: str = "bf16", single_core: bool = False,
          mode: str = "full", exp_sbuf: bool = True, enc_bufs: int = 8,
          tanh_bufs: int = 10, vdot_preload: bool = True,
          vdot_batch: bool = True, dma_rings: bool = False,
          dma_fuse: bool = False, layout: str = "sj", **sj_kw):
    if layout == "sj":
        return build_sj(repeat, main_dt=main_dt, single_core=single_core,
                        enc_bufs=enc_bufs, **sj_kw)
    enc_bufs = min(enc_bufs, 6)   # js SBUF budget (38 tanh bufs)
    """Build the per-core Bass module. `repeat` wraps the main compute in a
    For_i loop (used only by the benchmark harness to measure HW time by
    marginal wall-clock; the softmax tail + collective stay outside).
    mode: full | mm_only (perf experiment: main matmuls + dma only) |
    mm_tanh (mains + dma + tanh, no vdots/exps) | mm_resident (main
    matmuls only, enc preloaded to SBUF outside the loop)."""
    mm_only = mode in ("mm_only", "mm_resident", "mm_halfdma")
    mm_resident = mode == "mm_resident"
    half_dma = mode == "mm_halfdma"
    do_tanh = mode in ("full", "mm_tanh")
    do_vdot = mode == "full"
    if vdot_batch:
        # a full half's th tiles (32) stay alive until the burst, plus
        # the next half's first groups in flight
        tanh_bufs = max(tanh_bufs, 38)
    MD = {"f32r": F32R, "bf16": BF16}[main_dt]
    nc = bass.Bass("TRN2", target_bir_lowering=False, debug=False,
                   num_devices=1 if single_core else NCORES)

    # enc shard pre-tiled on host: [g, i, p, (k s)] (shared with the sj
    # layout; js views it back to [p, k, i, s] per group).
    encC = nc.dram_tensor("encC", [4, 2, 128, KC * SB], MD,
                          kind="ExternalInput").ap()
    w2t = nc.dram_tensor("w2t", [H, H], MD, kind="ExternalInput").ap()
    w1t = nc.dram_tensor("w1t", [2 * H // NCORES, H], F32R,
                         kind="ExternalInput").ap()
    hidT = nc.dram_tensor("hidT", [128, 16 // NCORES], F32R,
                          kind="ExternalInput").ap()
    bias = nc.dram_tensor("bias", [1, H], F32, kind="ExternalInput").ap()
    vwc = nc.dram_tensor("vwc", [128, JC], BF16, kind="ExternalInput").ap()
    # declared by both layouts so one in_map serves either build
    nc.dram_tensor("vrep", [128, H], BF16, kind="ExternalInput")
    out = nc.dram_tensor("out", [1, SL], F32, kind="ExternalOutput").ap()

    encC_v = encC.rearrange("g i p (k s) -> g p k i s", k=KC)
    w2t_v = w2t.rearrange("(k p) j -> p k j", p=128)     # [128, 8, 1024]
    w1t_v = w1t.rearrange("(k p) j -> p k j", p=128)     # [128, 2, 1024]

    with tile.TileContext(nc) as tc:
        with (
            tc.tile_pool(name="const", bufs=1) as const_pool,
            tc.tile_pool(name="enc", bufs=enc_bufs) as enc_pool,
            tc.tile_pool(name="tanh", bufs=tanh_bufs) as tanh_pool,
            tc.tile_pool(name="sm", bufs=1) as sm_pool,
            tc.tile_pool(name="pse", bufs=7, space="PSUM") as pse_pool,
            tc.tile_pool(name="psa", bufs=1, space="PSUM") as psa_pool,
            tc.tile_pool(name="dram", bufs=1, space="DRAM") as dram_pool,
        ):
            # --- tiny constants -------------------------------------------
            hid_sb = const_pool.tile([128, 16 // NCORES], F32R)
            nc.sync.dma_start(hid_sb[:], hidT[:])
            vw_sb = const_pool.tile([128, JC], BF16)
            nc.sync.dma_start(vw_sb[:], vwc[:])
            b_sb = const_pool.tile([1, H], F32)
            nc.sync.dma_start(b_sb[:], bias[:])

            # --- replicated weights: one tile per j-slab so the group-j
            # matmuls depend only on their own slab's DMA ---------------
            w2_tiles = []
            for j in range(JC):
                w2_j = const_pool.tile([128, KC, 128], MD, name=f"w2_{j}")
                nc.sync.dma_start(w2_j[:], w2t_v[:, :, j * 128:(j + 1) * 128])
                w2_tiles.append(w2_j)

            exps = sm_pool.tile([1, SL], F32)
            sums = sm_pool.tile([1, NSB], F32)

            # --- c0 = hidden @ W1T + attn_b (one row), sharded over cores
            c0_sb = const_pool.tile([128, JC], F32)

            NKC = 16 // NCORES   # local w1 chunks (c0 sharded over cores)

            def c0_section():
                w1_sb = const_pool.tile([128, NKC, H], F32R)
                nc.sync.dma_start(w1_sb[:], w1t_v[:])
                # bias arrives pre-divided by NCORES, so adding it to the
                # local partial and AllReduce-summing reconstructs c0+b
                part_row = const_pool.tile([1, H], F32)
                for half in range(2):
                    psum_c = pse_pool.tile([1, 512], F32, tag="pe",
                                           name="psum_c")
                    for kc in range(NKC):
                        nc.tensor.matmul(
                            psum_c[:],
                            hid_sb[:, kc:kc + 1],
                            w1_sb[:, kc, half * 512:(half + 1) * 512],
                            start=(kc == 0), stop=(kc == NKC - 1),
                        )
                    nc.vector.tensor_add(
                        part_row[:, half * 512:(half + 1) * 512],
                        psum_c[:],
                        b_sb[:, half * 512:(half + 1) * 512])
                ar_in = dram_pool.tile([1, H], F32)
                nc.gpsimd.dma_start(ar_in[:], part_row[:])
                if single_core:
                    ar_out = ar_in
                else:
                    ar_out = dram_pool.tile([1, H], F32)
                    nc.gpsimd.collective_compute(
                        "AllReduce",
                        mybir.AluOpType.add,
                        replica_groups=[list(range(NCORES))],
                        ins=[ar_in.opt()],
                        outs=[ar_out.opt()],
                    )
                nc.sync.dma_start(
                    c0_sb[:],
                    ar_out[:].rearrange("o (j p) -> (o p) j", p=128)
                )

            # --- main pipeline -------------------------------------------
            enc_res = [None]
            if mm_resident:
                enc_res[0] = const_pool.tile([128, 4, KC, 2, SB], MD,
                                             name="enc_res")
                for g in range(4):
                    nc.sync.dma_start(enc_res[0][:, g], encC_v[g])

            def main_body(_iv=None):
                # per j-group: 4 single-bank psum accumulators (the 4
                # s-blocks of the half), all fed k-outer so the 4 matmuls
                # of a (j, k) pair share one weight load. One [128, SB]
                # psum_a bank whose quadrant rows 0/32/64/96 hold the 4
                # s-blocks' logits so the 4 v-dots of a group land on
                # distinct PE column groups and stream concurrently.
                psum_a = [None]
                pending = []               # delayed v-dot emissions
                last_main = [None]         # latest main matmul instruction

                def flush():
                    for emit in pending:
                        emit()
                    pending.clear()

                def make_vdot(j, ths, pa):
                    def emit():
                        if vdot_preload:
                            # preload all 4 col-group weight slots first,
                            # then issue the 4 matmuls back-to-back so
                            # they stream concurrently (no interleaved
                            # LDW can stall the col-group pipeline; the
                            # per-MM auto-LDWs dedup against these).
                            # Pin each preload behind the latest main
                            # matmul so the scheduler cannot hoist it
                            # into an earlier weight-load's live range.
                            for q in range(4):
                                ldw = nc.tensor.ldweights(
                                    vw_sb[:, j:j + 1],
                                    tile_position=(0, 32 * q))
                                # mirror the fused matmul's rounded tile
                                # size so the per-MM auto-LDW dedups
                                # against this preload
                                ldw.ins.tile_size = (128, 32)
                                if last_main[0] is not None:
                                    bass._add_dep_helper(
                                        ldw.ins, last_main[0],
                                        sync=True,
                                        reason="pin vdot preload")
                        for q in range(4):
                            r = 32 * q
                            nc.tensor.matmul(
                                pa[r:r + 1, :],
                                vw_sb[:, j:j + 1], ths[q][:],
                                tile_position=(0, r),
                                start=(j == 0), stop=(j == JC - 1),
                            )
                    return emit

                def copy_logits(pa):
                    # DVE copies the logits bank to SBUF (~0.7us) so the
                    # psa bank frees fast and ACT's exps read SBUF off the
                    # PE-critical path (DVE is otherwise idle in-loop)
                    if not exp_sbuf:
                        return pa
                    lt = sm_pool.tile([128, SB], F32, tag="lt", name="lt",
                                      bufs=2)
                    nc.vector.tensor_copy(lt[:], pa[:])
                    return lt

                def emit_exps(h, lt):
                    for q in range(4):
                        sb = 4 * h + q
                        nc.scalar.activation(
                            exps[:, sb * SB:(sb + 1) * SB],
                            lt[32 * q:32 * q + 1, :], AF.Exp,
                            accum_out=sums[:, sb:sb + 1],
                        )

                prev_pa = None
                for h in range(2):
                    if mm_resident:
                        enc_ts = [enc_res[0][:, 2 * h + pp]
                                  for pp in range(2)]
                    elif dma_fuse:
                        # one 4MB DMA per half covering both s-block pairs
                        enc_t2 = enc_pool.tile([128, 2, KC, 2, SB], MD,
                                               tag="enc", bufs=enc_bufs // 2)
                        nc.sync.dma_start(
                            enc_t2[:],
                            encC.rearrange("g p x -> p g x")[:, 2 * h:2 * h + 2]
                            .rearrange("p g (k i s) -> p g k i s", k=KC, i=2),
                        )
                        enc_ts = [enc_t2[:, pp] for pp in range(2)]
                    else:
                        enc_ts = []
                        for pp in range(2):     # two s-block pairs per half
                            enc_t = enc_pool.tile([128, KC, 2, SB], MD,
                                                  tag="enc")
                            eng = (nc.scalar if (dma_rings and pp == 1)
                                   else nc.sync)
                            if half_dma:   # perf probe: half the bytes
                                eng.dma_start(enc_t[:, :KC // 2],
                                              encC_v[2 * h + pp][:, :KC // 2])
                            else:
                                eng.dma_start(enc_t[:], encC_v[2 * h + pp])
                            enc_ts.append(enc_t)
                    for j in range(JC):
                        pes = [
                            pse_pool.tile([128, SB], F32, tag="pe",
                                          name="pe")
                            for _ in range(4)
                        ]
                        for k in range(KC):
                            w = w2_tiles[j][:, k, :]
                            for q in range(4):
                                mm = nc.tensor.matmul(
                                    pes[q][:], w,
                                    enc_ts[q // 2][:, k, q % 2, :],
                                    start=(k == 0), stop=(k == KC - 1),
                                )
                                last_main[0] = mm.ins
                        if not do_tanh:
                            continue
                        if not vdot_batch or j == 0:
                            flush()
                        lt_prev = None
                        if do_vdot and j == 0:
                            # previous half's logits complete: DVE-copy
                            # them out before this half's first v-dots
                            # reuse the bank
                            if h == 1:
                                lt_prev = copy_logits(prev_pa)
                            psum_a[0] = psa_pool.tile(
                                [128, SB], F32, tag="psa", name="psa")
                        ths = []
                        for q in range(4):
                            th = tanh_pool.tile([128, SB], BF16,
                                                tag="th", name="th")
                            nc.scalar.activation(
                                th[:], pes[q][:], AF.Tanh,
                                bias=c0_sb[:, j:j + 1])
                            ths.append(th)
                        if lt_prev is not None:
                            # exps queue on ACT after this j's tanhs so
                            # they never delay the psum-bank recycle
                            emit_exps(0, lt_prev)
                        if do_vdot:
                            pending.append(make_vdot(j, ths, psum_a[0]))
                    prev_pa = psum_a[0]
                if do_vdot:
                    flush()
                    emit_exps(1, copy_logits(prev_pa))
                else:
                    nc.gpsimd.memset(exps[:], 1.0)
                    nc.gpsimd.memset(sums[:], 1.0)

            c0_section()
            if repeat == 1:
                main_body()
            else:
                with tc.For_i(0, repeat, 1,
                              hint_engines=(mybir.EngineType.PE,)) as _i:
                    main_body(_i)

            # --- softmax normalization across cores -----------------------
            if single_core:
                zg = sm_pool.tile([1, 1], F32)
                nc.vector.reduce_sum(zg[:], sums[:],
                                     axis=mybir.AxisListType.X)
            else:
                # AllGather the raw per-block sums (8 floats/core) and do a
                # single 64-element reduce afterwards
                ag_in = dram_pool.tile([1, NSB], F32)
                nc.gpsimd.dma_start(ag_in[:], sums[:])
                ag_out = dram_pool.tile([1, NCORES * NSB], F32)
                nc.gpsimd.collective_compute(
                    "AllGather",
                    mybir.AluOpType.bypass,
                    replica_groups=[list(range(NCORES))],
                    ins=[ag_in.opt()],
                    outs=[ag_out.opt()],
                )
                zs = sm_pool.tile([1, NCORES * NSB], F32)
                nc.gpsimd.dma_start(zs[:], ag_out[:])
                zg = sm_pool.tile([1, 1], F32)
                nc.vector.reduce_sum(zg[:], zs[:], axis=mybir.AxisListType.X)
            invz = sm_pool.tile([1, 1], F32)
            nc.vector.reciprocal(invz[:], zg[:])
            outv = sm_pool.tile([1, SL], F32)
            # split the 4096-element scale across ACT and DVE in parallel,
            # and ship each half as soon as it's done
            hl = SL // 2
            nc.scalar.activation(outv[:, :hl], exps[:, :hl], AF.Identity,
                                 scale=invz[:])
            nc.sync.dma_start(out[:, :hl], outv[:, :hl])
            nc.vector.tensor_scalar_mul(outv[:, hl:], exps[:, hl:], invz[:])
            nc.sync.dma_start(out[:, hl:], outv[:, hl:])

    _dedup_ldweights(nc)
    _split_multi_waits(nc)
    return nc


def prepare_in_maps(hidden, encoder_output, attn_w, attn_b, v_w,
                    main_dt="bf16"):
    hidden = np.asarray(hidden, dtype=np.float32)
    enc = np.asarray(encoder_output, dtype=np.float32)
    attn_w = np.asarray(attn_w, dtype=np.float32)
    attn_b = np.asarray(attn_b, dtype=np.float32)
    v_w = np.asarray(v_w, dtype=np.float32)

    import ml_dtypes
    md = np.float32 if main_dt == "f32r" else ml_dtypes.bfloat16
    w2t = np.ascontiguousarray(attn_w[:, 2 * H:].T).astype(md)   # [H, H]
    w1t_full = np.ascontiguousarray(attn_w[:, :2 * H].T)
    hidT_full = np.ascontiguousarray(hidden.reshape(16, 128).T)
    kpc = 16 // NCORES
    b = np.ascontiguousarray(attn_b.reshape(1, H)) / np.float32(NCORES)
    vwc = np.ascontiguousarray(v_w.reshape(JC, 128).T).astype(
        ml_dtypes.bfloat16)  # [128, 8]
    vrep = np.ascontiguousarray(
        np.broadcast_to(v_w.reshape(1, H), (128, H))).astype(
        ml_dtypes.bfloat16)  # [128, H] replicated

    in_maps = []
    for c in range(NCORES):
        encT = enc[c * SL:(c + 1) * SL, :].T.astype(md)   # [H, SL]
        # [g, i, p, k, s]: each (g, i) half contiguous per partition so
        # every half-tile DMA is 128 descriptors of 8KB
        encC = np.ascontiguousarray(
            encT.reshape(KC, 128, 4, 2, SB).transpose(2, 3, 1, 0, 4)
        ).reshape(4, 2, 128, KC * SB)
        in_maps.append({
            "encC": encC, "w2t": w2t,
            "w1t": np.ascontiguousarray(
                w1t_full[c * kpc * 128:(c + 1) * kpc * 128, :]),
            "hidT": np.ascontiguousarray(
                hidT_full[:, c * kpc:(c + 1) * kpc]),
            "bias": b, "vwc": vwc, "vrep": vrep,
        })
    return in_maps


_NC_CACHE = {}


def _get_nc(repeat: int = 1):
    if repeat not in _NC_CACHE:
        _NC_CACHE[repeat] = build(repeat)
    return _NC_CACHE[repeat]


def kernel(hidden, encoder_output, attn_w, attn_b, v_w):
    nc = _get_nc(1)
    in_maps = prepare_in_maps(hidden, encoder_output, attn_w, attn_b, v_w)
    res = run_bass_kernel_spmd(nc, in_maps, list(range(NCORES)))
    parts = []
    for c in range(NCORES):
        r = res.results[c]
        if "outT" in r:
            # outT[p, t] holds s_local = t*128 + p
            parts.append(np.ascontiguousarray(r["outT"].T).reshape(SL))
        else:
            parts.append(r["out"][0])
    return np.concatenate(parts)


# revision 60
# speedup vs baseline: 1.0291x; 1.0095x over previous
"""Trainium2 Bass kernel for the attention-MLP problem.

Reference computation (S=32768, H=1024):
    cat    = [broadcast(hidden, (S, 2H)) | encoder_output]   # [S, 3H]
    energy = tanh(cat @ attn_w.T + attn_b)                   # [S, H]
    logits = (energy @ v_w.T).squeeze()                      # [S]
    out    = softmax(logits)                                 # [S]

Because the hidden rows are identical, cat @ attn_w.T splits into
    c0  = hidden @ W1T + attn_b          (one row, [H])
    pre = enc @ W2T + c0                  (the real work)
with W1T = attn_w[:, :2H].T and W2T = attn_w[:, 2H:].T.

Sharding: seq axis split across 8 cores (4096 rows each); weights
replicated. Softmax normalization uses exp (no max subtraction needed:
|logits| <= ||v_w||_1 ~ 26, safely inside fp32 exp range) with an
AllGather of the 8 per-core partial sums.

Measured HW facts that shaped the design (marginal For_i benchmarks on
this part):
  * with all 8 cores busy the PE streams a 128x128x512 bf16 matmul in
    ~266-273ns (chip power-state downclock from the 1-core 222ns /
    2.4 GHz rate), so the 512 main matmuls floor at ~136-140us;
    LDWEIGHTS, semaphore updates, and satisfied waits are free in a
    back-to-back stream.
  * same-process decomposition of the previous [j, s] layout: pure
    mains 136us, +10us in-loop enc DMA (mostly bytes-proportional,
    i.e. physics), +1us tanh, +13us vdots/exps -> ~157-160us.

Default schedule (layout="sj", build_sj): energy computed in [s, j]
layout — enc chunks [128k, 128s] are the STATIONARY operand, W2 the
moving one — so the logits contraction over j runs along the free axis
and the entire v-dot disappears from the PE stream (PE does exactly the
512 main matmuls). Per s-tile chain: DVE adds c0 (replicated rows) to
the [128, 1024] psum, ACT tanh -> bf16, DVE multiply by v_rep + reduce
-> one logit per partition; one ACT exp per iteration over the [128,32]
logit tile; cross-partition normalization via a ones-vector matmul +
AllGather. Output is outT [128, 32] (host reassembles s = t*128 + p).
Other details:
  * enc shipped host-pre-tiled ([g, i, sq, p, k, s]); each tile
    arrives as eight contiguous quarter-DMAs (128 x 2KB descriptors
    each) so every s-tile's stationary data releases independently
    (split DMAs measured ~5us over whole-tile DMAs); 8 tile buffers of
    prefetch depth.
  * LDW dedup post-pass (_dedup_ldweights) drops the second auto-LDW of
    each (t, k) pair (LDWs are free anyway, this just shrinks streams).
  * tensor_tensor_reduce is broken on this walrus ("ISA wrong length"),
    hence the two-pass DVE mul + reduce.
  * bf16 operands (fp8 fails the 2e-2 tolerance: 7.1e-2 measured).
The previous [j, s] layout (energy^T in PSUM + PE v-dots + quadrant
tricks) is kept as build(layout="js") for comparison; it measures
~5-6us slower (156 vs 151us).
"""

import numpy as np

import concourse.bass as bass
import concourse.mybir as mybir
import concourse.tile as tile
from concourse.bass_utils import run_bass_kernel_spmd

H = 1024
S = 32768
NCORES = 8
SL = S // NCORES          # 4096 rows per core
SB = 512                  # seq block (columns of the psum tiles)
NSB = SL // SB            # 8 seq blocks per core
KC = H // 128             # 8 contraction chunks
JC = H // 128             # 8 output-row chunks

F32 = mybir.dt.float32
F32R = mybir.dt.float32r
BF16 = mybir.dt.bfloat16

AF = mybir.ActivationFunctionType


# ---------------------------------------------------------------------------
# Workaround for this walrus build: instructions only accept a single
# sync-wait command, but Tile can attach several. Hoist the extra waits
# onto NOPs inserted just before the instruction on the same engine
# (engines execute their stream in order, so semantics are preserved).
def _split_multi_waits(nc):
    end_bb = nc.cur_bb.bb
    for bb in nc.m.functions[0].blocks:
        insts = list(bb.instructions)
        out = []
        changed = False
        for inst in insts:
            si = inst.sync_info
            waits = list(si.on_wait) if si and si.on_wait else []
            if len(waits) > 1:
                changed = True
                for w in waits[:-1]:
                    nop = nc.engines[inst.engine].nop(nofuse=True).ins
                    end_bb.instructions.remove(nop)
                    nop.sync_info = mybir.SyncInfo(on_wait=[w], on_update=[])
                    out.append(nop)
                si.on_wait = waits[-1:]
            out.append(inst)
        if changed:
            bb.instructions = out
# ---------------------------------------------------------------------------


# Delete LDWEIGHTS that reload weights already resident in the PE array.
# Tile emits one InstLdweights per matmul; when the same stationary
# operand is already loaded at the same array position (and no
# intervening load clobbered its columns), the repeat is pure overhead.
# Position-aware: the array's 32-col strips hold independent weight sets
# (tile_position col tiling), so residency is tracked per column range —
# a new load only clobbers entries whose column ranges intersect.
# Matmuls never clobber loaded weights; fp32/fp32r matmuls self-load
# (clobber all); any other PE instruction conservatively resets tracking.
# Only sync-free LDWs are dropped.
def _dedup_ldweights(nc):
    n_dropped = 0
    for bb in nc.m.functions[0].blocks:
        out = []
        resident = {}   # col_start -> (col_end, key)
        for inst in bb.instructions:
            if inst.engine != mybir.EngineType.PE:
                out.append(inst)
                continue
            if isinstance(inst, mybir.InstLdweights):
                si = inst.sync_info
                has_sync = bool(si and (si.on_wait or si.on_update))
                pos = inst.tile_position or (0, 0)
                size = inst.tile_size or (128, 128)
                c0, c1 = pos[1], pos[1] + size[1]
                key = (
                    str(inst.ins[0]),
                    str(pos),
                    str(size),
                    str(inst.perf_mode),
                    str(inst.is_transpose),
                )
                if resident.get(c0) == (c1, key) and not has_sync:
                    n_dropped += 1
                    continue
                # clobber overlapping column ranges, then install
                resident = {s: (e, k) for s, (e, k) in resident.items()
                            if e <= c0 or s >= c1}
                resident[c0] = (c1, key)
            elif isinstance(inst, mybir.InstMatmult):
                w_dt = inst.ins[1].dtype if len(inst.ins) > 1 else None
                if w_dt in (mybir.dt.float32, mybir.dt.float32r):
                    resident = {}   # self-loading matmul clobbers array
            else:
                resident = {}
            out.append(inst)
        bb.instructions = out
    return n_dropped
# ---------------------------------------------------------------------------


def build_sj(repeat: int = 1, main_dt: str = "bf16",
             single_core: bool = False, enc_bufs: int = 8,
             use_ttr: bool = False, pre_bufs: int = 4, th_bufs: int = 4,
             prod_bufs: int = 3, pre_bf16: bool = False,
             chain: bool = True, dma_split='quarter'):
    """[s, j] energy layout: enc chunks are the stationary operand, W2 the
    moving one, so energy lands as [s-rows, j-cols] in PSUM and the whole
    v-dot disappears from the PE stream — DVE does (psum + c0_rep), ACT
    tanh, then DVE tensor_tensor_reduce(x v_rep, sum) produces one logit
    per partition. PE work: exactly the 512 main matmuls. Output is outT
    [128, SL/128] (host reassembles s = t*128 + p)."""
    MD = {"f32r": F32R, "bf16": BF16}[main_dt]
    NT = SL // 128            # 32 s-tiles per core
    nc = bass.Bass("TRN2", target_bir_lowering=False, debug=False,
                   num_devices=1 if single_core else NCORES)

    encC = nc.dram_tensor("encC", [4, 2, 4, 128, KC * 128], MD,
                          kind="ExternalInput").ap()
    w2t = nc.dram_tensor("w2t", [H, H], MD, kind="ExternalInput").ap()
    w1t = nc.dram_tensor("w1t", [2 * H // NCORES, H], F32R,
                         kind="ExternalInput").ap()
    hidT = nc.dram_tensor("hidT", [128, 16 // NCORES], F32R,
                          kind="ExternalInput").ap()
    bias = nc.dram_tensor("bias", [1, H], F32, kind="ExternalInput").ap()
    nc.dram_tensor("vwc", [128, JC], BF16, kind="ExternalInput")
    vrep = nc.dram_tensor("vrep", [128, H], BF16,
                          kind="ExternalInput").ap()
    outT = nc.dram_tensor("outT", [128, NT], F32,
                          kind="ExternalOutput").ap()

    # [g, i, sq, p, k, s128]: every (g, i, sq) quarter contiguous per
    # partition (128 descriptors of 2KB per quarter-DMA)
    encC_v = encC.rearrange("g i q p (k s) -> g i q p k s", k=KC)
    w2t_v = w2t.rearrange("(k p) j -> p k j", p=128)     # [128, 8, 1024]
    w1t_v = w1t.rearrange("(k p) j -> p k j", p=128)

    with tile.TileContext(nc) as tc:
        with (
            tc.tile_pool(name="const", bufs=1) as const_pool,
            tc.tile_pool(name="enc", bufs=enc_bufs) as enc_pool,
            tc.tile_pool(name="pre", bufs=pre_bufs) as pre_pool,
            tc.tile_pool(name="tanh", bufs=th_bufs) as tanh_pool,
            tc.tile_pool(name="prod", bufs=prod_bufs) as prod_pool,
            tc.tile_pool(name="sm", bufs=1) as sm_pool,
            tc.tile_pool(name="pse", bufs=4, space="PSUM") as pse_pool,
            tc.tile_pool(name="dram", bufs=1, space="DRAM") as dram_pool,
        ):
            hid_sb = const_pool.tile([128, 16 // NCORES], F32R)
            nc.sync.dma_start(hid_sb[:], hidT[:])
            vrep_sb = const_pool.tile([128, H], BF16)
            nc.sync.dma_start(vrep_sb[:], vrep[:])
            b_sb = const_pool.tile([1, H], F32)
            nc.sync.dma_start(b_sb[:], bias[:])
            w2r = const_pool.tile([128, KC, H], MD)
            nc.sync.dma_start(w2r[:], w2t_v[:])

            logits = sm_pool.tile([128, NT], F32)
            expst = sm_pool.tile([128, NT], F32)
            sumc = sm_pool.tile([128, 1], F32)
            c0_rep = const_pool.tile([128, H], F32)
            # ones row for PE-based partition broadcasts ([1,k]@[1,n] with
            # ones lhsT replicates a row across all 128 output partitions)
            ones_row = const_pool.tile([1, 128], F32)
            nc.gpsimd.memset(ones_row[:], 1.0)

            def bcast_rows(dst_sb, src_row, n):
                # dst_sb [128, n] <- broadcast of src_row [1, n]
                for o in range(0, n, 512):
                    w = min(512, n - o)
                    pb = pse_pool.tile([128, 512], F32, tag="pe",
                                       name="pb")
                    nc.tensor.matmul(pb[:, :w], ones_row[:],
                                     src_row[:, o:o + w],
                                     start=True, stop=True)
                    nc.vector.tensor_copy(dst_sb[:, o:o + w], pb[:, :w])

            NKC = 16 // NCORES

            def c0_section():
                w1_sb = const_pool.tile([128, NKC, H], F32R)
                nc.sync.dma_start(w1_sb[:], w1t_v[:])
                part_row = const_pool.tile([1, H], F32)
                for half in range(2):
                    psum_c = pse_pool.tile([1, 512], F32, tag="pe",
                                           name="psum_c")
                    for kc in range(NKC):
                        nc.tensor.matmul(
                            psum_c[:],
                            hid_sb[:, kc:kc + 1],
                            w1_sb[:, kc, half * 512:(half + 1) * 512],
                            start=(kc == 0), stop=(kc == NKC - 1),
                        )
                    nc.vector.tensor_add(
                        part_row[:, half * 512:(half + 1) * 512],
                        psum_c[:],
                        b_sb[:, half * 512:(half + 1) * 512])
                ar_in = dram_pool.tile([1, H], F32)
                nc.gpsimd.dma_start(ar_in[:], part_row[:])
                if single_core:
                    ar_out = ar_in
                else:
                    ar_out = dram_pool.tile([1, H], F32)
                    nc.gpsimd.collective_compute(
                        "AllReduce",
                        mybir.AluOpType.add,
                        replica_groups=[list(range(NCORES))],
                        ins=[ar_in.opt()],
                        outs=[ar_out.opt()],
                    )
                c0_row = const_pool.tile([1, H], F32)
                nc.sync.dma_start(c0_row[:], ar_out[:])
                bcast_rows(c0_rep, c0_row, H)

            def main_body(_iv=None):
                for h in range(2):
                    enc_ts = []
                    for pp in range(2):
                        enc_t = enc_pool.tile([128, 2, 4, KC, 128], MD,
                                              tag="enc")
                        if dma_split == "quarter":
                            # eight contiguous quarter-DMAs per tile:
                            # each s-tile's stationary data releases
                            # independently
                            for i in range(2):
                                for sq in range(4):
                                    nc.sync.dma_start(
                                        enc_t[:, i, sq],
                                        encC_v[2 * h + pp, i, sq])
                        elif dma_split:
                            # two contiguous half-DMAs per tile
                            for i in range(2):
                                nc.sync.dma_start(
                                    enc_t[:, i],
                                    encC_v[2 * h + pp, i]
                                    .rearrange("q p k s -> p q k s"))
                        else:
                            nc.sync.dma_start(
                                enc_t[:],
                                encC_v[2 * h + pp]
                                .rearrange("i q p k s -> p i q k s"))
                        enc_ts.append(enc_t)
                    for tl in range(16):       # s-tiles within the half
                        t = 16 * h + tl
                        q, off = tl // 4, (tl % 4) * 128
                        ps = pse_pool.tile([128, 2, 512], F32, tag="pe",
                                           name="ps")
                        for k in range(KC):
                            st = enc_ts[q // 2][:, q % 2, tl % 4, k, :]
                            for jh in range(2):
                                nc.tensor.matmul(
                                    ps[:, jh, :], st,
                                    w2r[:, k, jh * 512:(jh + 1) * 512],
                                    start=(k == 0), stop=(k == KC - 1),
                                )
                        if not chain:
                            continue
                        pre = pre_pool.tile([128, H],
                                            BF16 if pre_bf16 else F32,
                                            tag="pre", name="pre")
                        nc.vector.tensor_add(
                            pre[:],
                            ps[:].rearrange("p a b -> p (a b)"),
                            c0_rep[:])
                        th = tanh_pool.tile([128, H], BF16, tag="th",
                                            name="th")
                        nc.scalar.activation(th[:], pre[:], AF.Tanh)
                        prod = prod_pool.tile([128, H], BF16, tag="prod",
                                              name="prod")
                        if use_ttr:
                            nc.vector.tensor_tensor_reduce(
                                out=prod[:], in0=th[:], in1=vrep_sb[:],
                                scale=1.0, scalar=0.0,
                                op0=mybir.AluOpType.mult,
                                op1=mybir.AluOpType.add,
                                accum_out=logits[:, t:t + 1])
                        else:
                            nc.vector.tensor_mul(prod[:], th[:],
                                                  vrep_sb[:])
                            nc.vector.tensor_reduce(
                                logits[:, t:t + 1], prod[:],
                                axis=mybir.AxisListType.X,
                                op=mybir.AluOpType.add)
                # per-iteration: exp over all 32 logit columns
                if chain:
                    nc.scalar.activation(expst[:], logits[:], AF.Exp,
                                         accum_out=sumc[:])
                else:
                    nc.gpsimd.memset(expst[:], 1.0)
                    nc.gpsimd.memset(sumc[:], 1.0)

            c0_section()
            if repeat == 1:
                main_body()
            else:
                with tc.For_i(0, repeat, 1,
                              hint_engines=(mybir.EngineType.PE,)) as _i:
                    main_body(_i)

            # --- softmax normalization across cores -----------------------
            ones_sb = sm_pool.tile([128, 1], F32)
            nc.gpsimd.memset(ones_sb[:], 1.0)
            zp = pse_pool.tile([1, 1], F32, tag="pe", name="zp")
            nc.tensor.matmul(zp[:], ones_sb[:], sumc[:],
                             start=True, stop=True)
            if single_core:
                zg_src = zp
            else:
                ag_in = dram_pool.tile([1, 1], F32)
                zsb = sm_pool.tile([1, 1], F32)
                nc.vector.tensor_copy(zsb[:], zp[:])
                nc.gpsimd.dma_start(ag_in[:], zsb[:])
                ag_out = dram_pool.tile([1, NCORES], F32)
                nc.gpsimd.collective_compute(
                    "AllGather",
                    mybir.AluOpType.bypass,
                    replica_groups=[list(range(NCORES))],
                    ins=[ag_in.opt()],
                    outs=[ag_out.opt()],
                )
                zs = sm_pool.tile([1, NCORES], F32)
                nc.gpsimd.dma_start(zs[:], ag_out[:])
                zg_src = None
            zg = sm_pool.tile([1, 1], F32)
            if single_core:
                nc.vector.tensor_copy(zg[:], zg_src[:])
            else:
                nc.vector.reduce_sum(zg[:], zs[:], axis=mybir.AxisListType.X)
            invz = sm_pool.tile([1, 1], F32)
            nc.vector.reciprocal(invz[:], zg[:])
            invz_rep = sm_pool.tile([128, 1], F32)
            bcast_rows(invz_rep, invz, 1)
            outv = sm_pool.tile([128, NT], F32)
            nc.vector.tensor_scalar_mul(outv[:], expst[:], invz_rep[:])
            nc.sync.dma_start(outT[:], outv[:])

    _dedup_ldweights(nc)
    _split_multi_waits(nc)
    return nc


def build(repeat: int = 1, main_dt: str = "bf16", single_core: bool = False,
          mode: str = "full", exp_sbuf: bool = True, enc_bufs: int = 8,
          tanh_bufs: int = 10, vdot_preload: bool = True,
          vdot_batch: bool = True, dma_rings: bool = False,
          dma_fuse: bool = False, layout: str = "sj", **sj_kw):
    if layout == "sj":
        return build_sj(repeat, main_dt=main_dt, single_core=single_core,
                        enc_bufs=enc_bufs, **sj_kw)
    enc_bufs = min(enc_bufs, 6)   # js SBUF budget (38 tanh bufs)
    """Build the per-core Bass module. `repeat` wraps the main compute in a
    For_i loop (used only by the benchmark harness to measure HW time by
    marginal wall-clock; the softmax tail + collective stay outside).
    mode: full | mm_only (perf experiment: main matmuls + dma only) |
    mm_tanh (mains + dma + tanh, no vdots/exps) | mm_resident (main
    matmuls only, enc preloaded to SBUF outside the loop)."""
    mm_only = mode in ("mm_only", "mm_resident", "mm_halfdma")
    mm_resident = mode == "mm_resident"
    half_dma = mode == "mm_halfdma"
    do_tanh = mode in ("full", "mm_tanh")
    do_vdot = mode == "full"
    if vdot_batch:
        # a full half's th tiles (32) stay alive until the burst, plus
        # the next half's first groups in flight
        tanh_bufs = max(tanh_bufs, 38)
    MD = {"f32r": F32R, "bf16": BF16}[main_dt]
    nc = bass.Bass("TRN2", target_bir_lowering=False, debug=False,
                   num_devices=1 if single_core else NCORES)

    # enc shard pre-tiled on host: [g, i, sq, p, (k s)] (shared with the
    # sj layout; js views it back to [p, k, i, s] per group).
    encC = nc.dram_tensor("encC", [4, 2, 4, 128, KC * 128], MD,
                          kind="ExternalInput").ap()
    w2t = nc.dram_tensor("w2t", [H, H], MD, kind="ExternalInput").ap()
    w1t = nc.dram_tensor("w1t", [2 * H // NCORES, H], F32R,
                         kind="ExternalInput").ap()
    hidT = nc.dram_tensor("hidT", [128, 16 // NCORES], F32R,
                          kind="ExternalInput").ap()
    bias = nc.dram_tensor("bias", [1, H], F32, kind="ExternalInput").ap()
    vwc = nc.dram_tensor("vwc", [128, JC], BF16, kind="ExternalInput").ap()
    # declared by both layouts so one in_map serves either build
    nc.dram_tensor("vrep", [128, H], BF16, kind="ExternalInput")
    out = nc.dram_tensor("out", [1, SL], F32, kind="ExternalOutput").ap()

    encC_v = encC.rearrange("g i q p (k s) -> g p k i q s", k=KC)
    w2t_v = w2t.rearrange("(k p) j -> p k j", p=128)     # [128, 8, 1024]
    w1t_v = w1t.rearrange("(k p) j -> p k j", p=128)     # [128, 2, 1024]

    with tile.TileContext(nc) as tc:
        with (
            tc.tile_pool(name="const", bufs=1) as const_pool,
            tc.tile_pool(name="enc", bufs=enc_bufs) as enc_pool,
            tc.tile_pool(name="tanh", bufs=tanh_bufs) as tanh_pool,
            tc.tile_pool(name="sm", bufs=1) as sm_pool,
            tc.tile_pool(name="pse", bufs=7, space="PSUM") as pse_pool,
            tc.tile_pool(name="psa", bufs=1, space="PSUM") as psa_pool,
            tc.tile_pool(name="dram", bufs=1, space="DRAM") as dram_pool,
        ):
            # --- tiny constants -------------------------------------------
            hid_sb = const_pool.tile([128, 16 // NCORES], F32R)
            nc.sync.dma_start(hid_sb[:], hidT[:])
            vw_sb = const_pool.tile([128, JC], BF16)
            nc.sync.dma_start(vw_sb[:], vwc[:])
            b_sb = const_pool.tile([1, H], F32)
            nc.sync.dma_start(b_sb[:], bias[:])

            # --- replicated weights: one tile per j-slab so the group-j
            # matmuls depend only on their own slab's DMA ---------------
            w2_tiles = []
            for j in range(JC):
                w2_j = const_pool.tile([128, KC, 128], MD, name=f"w2_{j}")
                nc.sync.dma_start(w2_j[:], w2t_v[:, :, j * 128:(j + 1) * 128])
                w2_tiles.append(w2_j)

            exps = sm_pool.tile([1, SL], F32)
            sums = sm_pool.tile([1, NSB], F32)

            # --- c0 = hidden @ W1T + attn_b (one row), sharded over cores
            c0_sb = const_pool.tile([128, JC], F32)

            NKC = 16 // NCORES   # local w1 chunks (c0 sharded over cores)

            def c0_section():
                w1_sb = const_pool.tile([128, NKC, H], F32R)
                nc.sync.dma_start(w1_sb[:], w1t_v[:])
                # bias arrives pre-divided by NCORES, so adding it to the
                # local partial and AllReduce-summing reconstructs c0+b
                part_row = const_pool.tile([1, H], F32)
                for half in range(2):
                    psum_c = pse_pool.tile([1, 512], F32, tag="pe",
                                           name="psum_c")
                    for kc in range(NKC):
                        nc.tensor.matmul(
                            psum_c[:],
                            hid_sb[:, kc:kc + 1],
                            w1_sb[:, kc, half * 512:(half + 1) * 512],
                            start=(kc == 0), stop=(kc == NKC - 1),
                        )
                    nc.vector.tensor_add(
                        part_row[:, half * 512:(half + 1) * 512],
                        psum_c[:],
                        b_sb[:, half * 512:(half + 1) * 512])
                ar_in = dram_pool.tile([1, H], F32)
                nc.gpsimd.dma_start(ar_in[:], part_row[:])
                if single_core:
                    ar_out = ar_in
                else:
                    ar_out = dram_pool.tile([1, H], F32)
                    nc.gpsimd.collective_compute(
                        "AllReduce",
                        mybir.AluOpType.add,
                        replica_groups=[list(range(NCORES))],
                        ins=[ar_in.opt()],
                        outs=[ar_out.opt()],
                    )
                nc.sync.dma_start(
                    c0_sb[:],
                    ar_out[:].rearrange("o (j p) -> (o p) j", p=128)
                )

            # --- main pipeline -------------------------------------------
            enc_res = [None]
            if mm_resident:
                enc_res[0] = const_pool.tile([128, 4, KC, 2, SB], MD,
                                             name="enc_res")
                for g in range(4):
                    nc.sync.dma_start(enc_res[0][:, g], encC_v[g])

            def main_body(_iv=None):
                # per j-group: 4 single-bank psum accumulators (the 4
                # s-blocks of the half), all fed k-outer so the 4 matmuls
                # of a (j, k) pair share one weight load. One [128, SB]
                # psum_a bank whose quadrant rows 0/32/64/96 hold the 4
                # s-blocks' logits so the 4 v-dots of a group land on
                # distinct PE column groups and stream concurrently.
                psum_a = [None]
                pending = []               # delayed v-dot emissions
                last_main = [None]         # latest main matmul instruction

                def flush():
                    for emit in pending:
                        emit()
                    pending.clear()

                def make_vdot(j, ths, pa):
                    def emit():
                        if vdot_preload:
                            # preload all 4 col-group weight slots first,
                            # then issue the 4 matmuls back-to-back so
                            # they stream concurrently (no interleaved
                            # LDW can stall the col-group pipeline; the
                            # per-MM auto-LDWs dedup against these).
                            # Pin each preload behind the latest main
                            # matmul so the scheduler cannot hoist it
                            # into an earlier weight-load's live range.
                            for q in range(4):
                                ldw = nc.tensor.ldweights(
                                    vw_sb[:, j:j + 1],
                                    tile_position=(0, 32 * q))
                                # mirror the fused matmul's rounded tile
                                # size so the per-MM auto-LDW dedups
                                # against this preload
                                ldw.ins.tile_size = (128, 32)
                                if last_main[0] is not None:
                                    bass._add_dep_helper(
                                        ldw.ins, last_main[0],
                                        sync=True,
                                        reason="pin vdot preload")
                        for q in range(4):
                            r = 32 * q
                            nc.tensor.matmul(
                                pa[r:r + 1, :],
                                vw_sb[:, j:j + 1], ths[q][:],
                                tile_position=(0, r),
                                start=(j == 0), stop=(j == JC - 1),
                            )
                    return emit

                def copy_logits(pa):
                    # DVE copies the logits bank to SBUF (~0.7us) so the
                    # psa bank frees fast and ACT's exps read SBUF off the
                    # PE-critical path (DVE is otherwise idle in-loop)
                    if not exp_sbuf:
                        return pa
                    lt = sm_pool.tile([128, SB], F32, tag="lt", name="lt",
                                      bufs=2)
                    nc.vector.tensor_copy(lt[:], pa[:])
                    return lt

                def emit_exps(h, lt):
                    for q in range(4):
                        sb = 4 * h + q
                        nc.scalar.activation(
                            exps[:, sb * SB:(sb + 1) * SB],
                            lt[32 * q:32 * q + 1, :], AF.Exp,
                            accum_out=sums[:, sb:sb + 1],
                        )

                prev_pa = None
                for h in range(2):
                    if mm_resident:
                        enc_ts = [enc_res[0][:, 2 * h + pp]
                                  for pp in range(2)]
                    elif dma_fuse:
                        # one 4MB DMA per half covering both s-block pairs
                        enc_t2 = enc_pool.tile([128, 2, KC, 2, SB], MD,
                                               tag="enc", bufs=enc_bufs // 2)
                        nc.sync.dma_start(
                            enc_t2[:],
                            encC.rearrange("g p x -> p g x")[:, 2 * h:2 * h + 2]
                            .rearrange("p g (k i s) -> p g k i s", k=KC, i=2),
                        )
                        enc_ts = [enc_t2[:, pp] for pp in range(2)]
                    else:
                        enc_ts = []
                        for pp in range(2):     # two s-block pairs per half
                            enc_t = enc_pool.tile([128, KC, 2, SB], MD,
                                                  tag="enc")
                            eng = (nc.scalar if (dma_rings and pp == 1)
                                   else nc.sync)
                            if half_dma:   # perf probe: half the bytes
                                eng.dma_start(enc_t[:, :KC // 2],
                                              encC_v[2 * h + pp][:, :KC // 2])
                            else:
                                eng.dma_start(enc_t[:], encC_v[2 * h + pp])
                            enc_ts.append(enc_t)
                    for j in range(JC):
                        pes = [
                            pse_pool.tile([128, SB], F32, tag="pe",
                                          name="pe")
                            for _ in range(4)
                        ]
                        for k in range(KC):
                            w = w2_tiles[j][:, k, :]
                            for q in range(4):
                                mm = nc.tensor.matmul(
                                    pes[q][:], w,
                                    enc_ts[q // 2][:, k, q % 2, :],
                                    start=(k == 0), stop=(k == KC - 1),
                                )
                                last_main[0] = mm.ins
                        if not do_tanh:
                            continue
                        if not vdot_batch or j == 0:
                            flush()
                        lt_prev = None
                        if do_vdot and j == 0:
                            # previous half's logits complete: DVE-copy
                            # them out before this half's first v-dots
                            # reuse the bank
                            if h == 1:
                                lt_prev = copy_logits(prev_pa)
                            psum_a[0] = psa_pool.tile(
                                [128, SB], F32, tag="psa", name="psa")
                        ths = []
                        for q in range(4):
                            th = tanh_pool.tile([128, SB], BF16,
                                                tag="th", name="th")
                            nc.scalar.activation(
                                th[:], pes[q][:], AF.Tanh,
                                bias=c0_sb[:, j:j + 1])
                            ths.append(th)
                        if lt_prev is not None:
                            # exps queue on ACT after this j's tanhs so
                            # they never delay the psum-bank recycle
                            emit_exps(0, lt_prev)
                        if do_vdot:
                            pending.append(make_vdot(j, ths, psum_a[0]))
                    prev_pa = psum_a[0]
                if do_vdot:
                    flush()
                    emit_exps(1, copy_logits(prev_pa))
                else:
                    nc.gpsimd.memset(exps[:], 1.0)
                    nc.gpsimd.memset(sums[:], 1.0)

            c0_section()
            if repeat == 1:
                main_body()
            else:
                with tc.For_i(0, repeat, 1,
                              hint_engines=(mybir.EngineType.PE,)) as _i:
                    main_body(_i)

            # --- softmax normalization across cores -----------------------
            if single_core:
                zg = sm_pool.tile([1, 1], F32)
                nc.vector.reduce_sum(zg[:], sums[:],
                                     axis=mybir.AxisListType.X)
            else:
                # AllGather the raw per-block sums (8 floats/core) and do a
                # single 64-element reduce afterwards
                ag_in = dram_pool.tile([1, NSB], F32)
                nc.gpsimd.dma_start(ag_in[:], sums[:])
                ag_out = dram_pool.tile([1, NCORES * NSB], F32)
                nc.gpsimd.collective_compute(
                    "AllGather",
                    mybir.AluOpType.bypass,
                    replica_groups=[list(range(NCORES))],
                    ins=[ag_in.opt()],
                    outs=[ag_out.opt()],
                )
                zs = sm_pool.tile([1, NCORES * NSB], F32)
                nc.gpsimd.dma_start(zs[:], ag_out[:])
                zg = sm_pool.tile([1, 1], F32)
                nc.vector.reduce_sum(zg[:], zs[:], axis=mybir.AxisListType.X)
            invz = sm_pool.tile([1, 1], F32)
            nc.vector.reciprocal(invz[:], zg[:])
            outv = sm_pool.tile([1, SL], F32)
            # split the 4096-element scale across ACT and DVE in parallel,
            # and ship each half as soon as it's done
            hl = SL // 2
            nc.scalar.activation(outv[:, :hl], exps[:, :hl], AF.Identity,
                                 scale=invz[:])
            nc.sync.dma_start(out[:, :hl], outv[:, :hl])
            nc.vector.tensor_scalar_mul(outv[:, hl:], exps[:, hl:], invz[:])
            nc.sync.dma_start(out[:, hl:], outv[:, hl:])

    _dedup_ldweights(nc)
    _split_multi_waits(nc)
    return nc


def prepare_in_maps(hidden, encoder_output, attn_w, attn_b, v_w,
                    main_dt="bf16"):
    hidden = np.asarray(hidden, dtype=np.float32)
    enc = np.asarray(encoder_output, dtype=np.float32)
    attn_w = np.asarray(attn_w, dtype=np.float32)
    attn_b = np.asarray(attn_b, dtype=np.float32)
    v_w = np.asarray(v_w, dtype=np.float32)

    import ml_dtypes
    md = np.float32 if main_dt == "f32r" else ml_dtypes.bfloat16
    w2t = np.ascontiguousarray(attn_w[:, 2 * H:].T).astype(md)   # [H, H]
    w1t_full = np.ascontiguousarray(attn_w[:, :2 * H].T)
    hidT_full = np.ascontiguousarray(hidden.reshape(16, 128).T)
    kpc = 16 // NCORES
    b = np.ascontiguousarray(attn_b.reshape(1, H)) / np.float32(NCORES)
    vwc = np.ascontiguousarray(v_w.reshape(JC, 128).T).astype(
        ml_dtypes.bfloat16)  # [128, 8]
    vrep = np.ascontiguousarray(
        np.broadcast_to(v_w.reshape(1, H), (128, H))).astype(
        ml_dtypes.bfloat16)  # [128, H] replicated

    in_maps = []
    for c in range(NCORES):
        encT = enc[c * SL:(c + 1) * SL, :].T.astype(md)   # [H, SL]
        # [g, i, sq, p, k, s128]: every (g, i, sq) quarter contiguous
        # per partition so each quarter-tile DMA is 128 descriptors of 2KB
        encC = np.ascontiguousarray(
            encT.reshape(KC, 128, 4, 2, 4, 128).transpose(2, 3, 4, 1, 0, 5)
        ).reshape(4, 2, 4, 128, KC * 128)
        in_maps.append({
            "encC": encC, "w2t": w2t,
            "w1t": np.ascontiguousarray(
                w1t_full[c * kpc * 128:(c + 1) * kpc * 128, :]),
            "hidT": np.ascontiguousarray(
                hidT_full[:, c * kpc:(c + 1) * kpc]),
            "bias": b, "vwc": vwc, "vrep": vrep,
        })
    return in_maps


_NC_CACHE = {}


def _get_nc(repeat: int = 1):
    if repeat not in _NC_CACHE:
        _NC_CACHE[repeat] = build(repeat)
    return _NC_CACHE[repeat]


def kernel(hidden, encoder_output, attn_w, attn_b, v_w):
    nc = _get_nc(1)
    in_maps = prepare_in_maps(hidden, encoder_output, attn_w, attn_b, v_w)
    res = run_bass_kernel_spmd(nc, in_maps, list(range(NCORES)))
    parts = []
    for c in range(NCORES):
        r = res.results[c]
        if "outT" in r:
            # outT[p, t] holds s_local = t*128 + p
            parts.append(np.ascontiguousarray(r["outT"].T).reshape(SL))
        else:
            parts.append(r["out"][0])
    return np.concatenate(parts)


# revision 65
# speedup vs baseline: 1.0413x; 1.0119x over previous
"""Trainium2 Bass kernel for the attention-MLP problem.

Reference computation (S=32768, H=1024):
    cat    = [broadcast(hidden, (S, 2H)) | encoder_output]   # [S, 3H]
    energy = tanh(cat @ attn_w.T + attn_b)                   # [S, H]
    logits = (energy @ v_w.T).squeeze()                      # [S]
    out    = softmax(logits)                                 # [S]

Because the hidden rows are identical, cat @ attn_w.T splits into
    c0  = hidden @ W1T + attn_b          (one row, [H])
    pre = enc @ W2T + c0                  (the real work)
with W1T = attn_w[:, :2H].T and W2T = attn_w[:, 2H:].T.

Sharding: seq axis split across 8 cores (4096 rows each); weights
replicated. Softmax normalization uses exp (no max subtraction needed:
|logits| <= ||v_w||_1 ~ 26, safely inside fp32 exp range) with an
AllGather of the 8 per-core partial sums.

Measured HW facts that shaped the design (marginal For_i benchmarks on
this part):
  * with all 8 cores busy the PE streams a 128x128x512 bf16 matmul in
    ~266-273ns (chip power-state downclock from the 1-core 222ns /
    2.4 GHz rate), so the 512 main matmuls floor at ~136-140us;
    LDWEIGHTS, semaphore updates, and satisfied waits are free in a
    back-to-back stream.
  * same-process decomposition of the previous [j, s] layout: pure
    mains 136us, +10us in-loop enc DMA (mostly bytes-proportional,
    i.e. physics), +1us tanh, +13us vdots/exps -> ~157-160us.

Default schedule (layout="sj", build_sj): energy computed in [s, j]
layout — enc chunks [128k, 128s] are the STATIONARY operand, W2 the
moving one — so the logits contraction over j runs along the free axis
and the entire v-dot disappears from the PE stream (PE does exactly the
512 main matmuls). Per s-tile chain: DVE adds c0 (replicated rows) to
the [128, 1024] psum, ACT tanh -> bf16, DVE multiply by v_rep + reduce
-> one logit per partition; one ACT exp per iteration over the [128,32]
logit tile; cross-partition normalization via a ones-vector matmul +
AllGather. Output is outT [128, 32] (host reassembles s = t*128 + p).
Other details:
  * enc shipped host-pre-tiled ([g, i, sq, p, k, s]); each tile
    arrives as 32 contiguous k-quarter DMAs (128 x 512B descriptors
    each) so every s-tile's stationary data releases independently and
    its first k-groups start as soon as their slice lands (split DMAs
    measured ~7us total over whole-tile DMAs, positive at every
    granularity step); 8 tile buffers of prefetch depth.
  * LDW dedup post-pass (_dedup_ldweights) drops the second auto-LDW of
    each (t, k) pair (LDWs are free anyway, this just shrinks streams).
  * tensor_tensor_reduce is broken on this walrus ("ISA wrong length"),
    hence the two-pass DVE mul + reduce.
  * bf16 operands (fp8 fails the 2e-2 tolerance: 7.1e-2 measured).
The previous [j, s] layout (energy^T in PSUM + PE v-dots + quadrant
tricks) is kept as build(layout="js") for comparison; it measures
~5-6us slower (156 vs 151us).
"""

import numpy as np

import concourse.bass as bass
import concourse.mybir as mybir
import concourse.tile as tile
from concourse.bass_utils import run_bass_kernel_spmd

H = 1024
S = 32768
NCORES = 8
SL = S // NCORES          # 4096 rows per core
SB = 512                  # seq block (columns of the psum tiles)
NSB = SL // SB            # 8 seq blocks per core
KC = H // 128             # 8 contraction chunks
JC = H // 128             # 8 output-row chunks

F32 = mybir.dt.float32
F32R = mybir.dt.float32r
BF16 = mybir.dt.bfloat16

AF = mybir.ActivationFunctionType


# ---------------------------------------------------------------------------
# Workaround for this walrus build: instructions only accept a single
# sync-wait command, but Tile can attach several. Hoist the extra waits
# onto NOPs inserted just before the instruction on the same engine
# (engines execute their stream in order, so semantics are preserved).
def _split_multi_waits(nc):
    end_bb = nc.cur_bb.bb
    for bb in nc.m.functions[0].blocks:
        insts = list(bb.instructions)
        out = []
        changed = False
        for inst in insts:
            si = inst.sync_info
            waits = list(si.on_wait) if si and si.on_wait else []
            if len(waits) > 1:
                changed = True
                for w in waits[:-1]:
                    nop = nc.engines[inst.engine].nop(nofuse=True).ins
                    end_bb.instructions.remove(nop)
                    nop.sync_info = mybir.SyncInfo(on_wait=[w], on_update=[])
                    out.append(nop)
                si.on_wait = waits[-1:]
            out.append(inst)
        if changed:
            bb.instructions = out
# ---------------------------------------------------------------------------


# Delete LDWEIGHTS that reload weights already resident in the PE array.
# Tile emits one InstLdweights per matmul; when the same stationary
# operand is already loaded at the same array position (and no
# intervening load clobbered its columns), the repeat is pure overhead.
# Position-aware: the array's 32-col strips hold independent weight sets
# (tile_position col tiling), so residency is tracked per column range —
# a new load only clobbers entries whose column ranges intersect.
# Matmuls never clobber loaded weights; fp32/fp32r matmuls self-load
# (clobber all); any other PE instruction conservatively resets tracking.
# Only sync-free LDWs are dropped.
def _dedup_ldweights(nc):
    n_dropped = 0
    for bb in nc.m.functions[0].blocks:
        out = []
        resident = {}   # col_start -> (col_end, key)
        for inst in bb.instructions:
            if inst.engine != mybir.EngineType.PE:
                out.append(inst)
                continue
            if isinstance(inst, mybir.InstLdweights):
                si = inst.sync_info
                has_sync = bool(si and (si.on_wait or si.on_update))
                pos = inst.tile_position or (0, 0)
                size = inst.tile_size or (128, 128)
                c0, c1 = pos[1], pos[1] + size[1]
                key = (
                    str(inst.ins[0]),
                    str(pos),
                    str(size),
                    str(inst.perf_mode),
                    str(inst.is_transpose),
                )
                if resident.get(c0) == (c1, key) and not has_sync:
                    n_dropped += 1
                    continue
                # clobber overlapping column ranges, then install
                resident = {s: (e, k) for s, (e, k) in resident.items()
                            if e <= c0 or s >= c1}
                resident[c0] = (c1, key)
            elif isinstance(inst, mybir.InstMatmult):
                w_dt = inst.ins[1].dtype if len(inst.ins) > 1 else None
                if w_dt in (mybir.dt.float32, mybir.dt.float32r):
                    resident = {}   # self-loading matmul clobbers array
            else:
                resident = {}
            out.append(inst)
        bb.instructions = out
    return n_dropped
# ---------------------------------------------------------------------------


def build_sj(repeat: int = 1, main_dt: str = "bf16",
             single_core: bool = False, enc_bufs: int = 8,
             use_ttr: bool = False, pre_bufs: int = 4, th_bufs: int = 4,
             prod_bufs: int = 3, pre_bf16: bool = False,
             chain: bool = True, dma_split='sixteenth',
             dma_rings: bool = False):
    """[s, j] energy layout: enc chunks are the stationary operand, W2 the
    moving one, so energy lands as [s-rows, j-cols] in PSUM and the whole
    v-dot disappears from the PE stream — DVE does (psum + c0_rep), ACT
    tanh, then DVE tensor_tensor_reduce(x v_rep, sum) produces one logit
    per partition. PE work: exactly the 512 main matmuls. Output is outT
    [128, SL/128] (host reassembles s = t*128 + p)."""
    MD = {"f32r": F32R, "bf16": BF16}[main_dt]
    NT = SL // 128            # 32 s-tiles per core
    nc = bass.Bass("TRN2", target_bir_lowering=False, debug=False,
                   num_devices=1 if single_core else NCORES)

    encC = nc.dram_tensor("encC", [4, 2, 4, 128, KC * 128], MD,
                          kind="ExternalInput").ap()
    w2t = nc.dram_tensor("w2t", [H, H], MD, kind="ExternalInput").ap()
    w1t = nc.dram_tensor("w1t", [2 * H // NCORES, H], F32R,
                         kind="ExternalInput").ap()
    hidT = nc.dram_tensor("hidT", [128, 16 // NCORES], F32R,
                          kind="ExternalInput").ap()
    bias = nc.dram_tensor("bias", [1, H], F32, kind="ExternalInput").ap()
    nc.dram_tensor("vwc", [128, JC], BF16, kind="ExternalInput")
    vrep = nc.dram_tensor("vrep", [128, H], BF16,
                          kind="ExternalInput").ap()
    outT = nc.dram_tensor("outT", [128, NT], F32,
                          kind="ExternalOutput").ap()

    # [g, i, sq, p, k, s128]: every (g, i, sq) quarter contiguous per
    # partition (128 descriptors of 2KB per quarter-DMA)
    encC_v = encC.rearrange("g i q p (k s) -> g i q p k s", k=KC)
    w2t_v = w2t.rearrange("(k p) j -> p k j", p=128)     # [128, 8, 1024]
    w1t_v = w1t.rearrange("(k p) j -> p k j", p=128)

    with tile.TileContext(nc) as tc:
        with (
            tc.tile_pool(name="const", bufs=1) as const_pool,
            tc.tile_pool(name="enc", bufs=enc_bufs) as enc_pool,
            tc.tile_pool(name="pre", bufs=pre_bufs) as pre_pool,
            tc.tile_pool(name="tanh", bufs=th_bufs) as tanh_pool,
            tc.tile_pool(name="prod", bufs=prod_bufs) as prod_pool,
            tc.tile_pool(name="sm", bufs=1) as sm_pool,
            tc.tile_pool(name="pse", bufs=4, space="PSUM") as pse_pool,
            tc.tile_pool(name="dram", bufs=1, space="DRAM") as dram_pool,
        ):
            hid_sb = const_pool.tile([128, 16 // NCORES], F32R)
            nc.sync.dma_start(hid_sb[:], hidT[:])
            vrep_sb = const_pool.tile([128, H], BF16)
            nc.sync.dma_start(vrep_sb[:], vrep[:])
            b_sb = const_pool.tile([1, H], F32)
            nc.sync.dma_start(b_sb[:], bias[:])
            w2r = const_pool.tile([128, KC, H], MD)
            nc.sync.dma_start(w2r[:], w2t_v[:])

            logits = sm_pool.tile([128, NT], F32)
            expst = sm_pool.tile([128, NT], F32)
            sumc = sm_pool.tile([128, 1], F32)
            c0_rep = const_pool.tile([128, H], F32)
            # ones row for PE-based partition broadcasts ([1,k]@[1,n] with
            # ones lhsT replicates a row across all 128 output partitions)
            ones_row = const_pool.tile([1, 128], F32)
            nc.gpsimd.memset(ones_row[:], 1.0)

            def bcast_rows(dst_sb, src_row, n):
                # dst_sb [128, n] <- broadcast of src_row [1, n]
                for o in range(0, n, 512):
                    w = min(512, n - o)
                    pb = pse_pool.tile([128, 512], F32, tag="pe",
                                       name="pb")
                    nc.tensor.matmul(pb[:, :w], ones_row[:],
                                     src_row[:, o:o + w],
                                     start=True, stop=True)
                    nc.vector.tensor_copy(dst_sb[:, o:o + w], pb[:, :w])

            NKC = 16 // NCORES

            def c0_section():
                w1_sb = const_pool.tile([128, NKC, H], F32R)
                nc.sync.dma_start(w1_sb[:], w1t_v[:])
                part_row = const_pool.tile([1, H], F32)
                for half in range(2):
                    psum_c = pse_pool.tile([1, 512], F32, tag="pe",
                                           name="psum_c")
                    for kc in range(NKC):
                        nc.tensor.matmul(
                            psum_c[:],
                            hid_sb[:, kc:kc + 1],
                            w1_sb[:, kc, half * 512:(half + 1) * 512],
                            start=(kc == 0), stop=(kc == NKC - 1),
                        )
                    nc.vector.tensor_add(
                        part_row[:, half * 512:(half + 1) * 512],
                        psum_c[:],
                        b_sb[:, half * 512:(half + 1) * 512])
                ar_in = dram_pool.tile([1, H], F32)
                nc.gpsimd.dma_start(ar_in[:], part_row[:])
                if single_core:
                    ar_out = ar_in
                else:
                    ar_out = dram_pool.tile([1, H], F32)
                    nc.gpsimd.collective_compute(
                        "AllReduce",
                        mybir.AluOpType.add,
                        replica_groups=[list(range(NCORES))],
                        ins=[ar_in.opt()],
                        outs=[ar_out.opt()],
                    )
                c0_row = const_pool.tile([1, H], F32)
                nc.sync.dma_start(c0_row[:], ar_out[:])
                bcast_rows(c0_rep, c0_row, H)

            def main_body(_iv=None):
                for h in range(2):
                    enc_ts = []
                    for pp in range(2):
                        enc_t = enc_pool.tile([128, 2, 4, KC, 128], MD,
                                              tag="enc")
                        if dma_split == "sixteenth":
                            # 32 k-quarter DMAs per tile
                            for i in range(2):
                                for sq in range(4):
                                    for kq in range(4):
                                        ks = slice(kq * (KC // 4),
                                                   (kq + 1) * (KC // 4))
                                        nc.sync.dma_start(
                                            enc_t[:, i, sq, ks],
                                            encC_v[2 * h + pp, i, sq]
                                            [:, ks])
                        elif dma_split == "eighth":
                            # sixteen contiguous k-half DMAs per tile:
                            # an s-tile's first k-groups start on its
                            # first half (k-major within the quarter)
                            for i in range(2):
                                for sq in range(4):
                                    for kh in range(2):
                                        ks = slice(kh * (KC // 2),
                                                   (kh + 1) * (KC // 2))
                                        nc.sync.dma_start(
                                            enc_t[:, i, sq, ks],
                                            encC_v[2 * h + pp, i, sq]
                                            [:, ks])
                        elif dma_split == "quarter":
                            # eight contiguous quarter-DMAs per tile:
                            # each s-tile's stationary data releases
                            # independently
                            for i in range(2):
                                for sq in range(4):
                                    eng = (nc.scalar
                                           if (dma_rings and sq % 2)
                                           else nc.sync)
                                    eng.dma_start(
                                        enc_t[:, i, sq],
                                        encC_v[2 * h + pp, i, sq])
                        elif dma_split:
                            # two contiguous half-DMAs per tile
                            for i in range(2):
                                nc.sync.dma_start(
                                    enc_t[:, i],
                                    encC_v[2 * h + pp, i]
                                    .rearrange("q p k s -> p q k s"))
                        else:
                            nc.sync.dma_start(
                                enc_t[:],
                                encC_v[2 * h + pp]
                                .rearrange("i q p k s -> p i q k s"))
                        enc_ts.append(enc_t)
                    for tl in range(16):       # s-tiles within the half
                        t = 16 * h + tl
                        q, off = tl // 4, (tl % 4) * 128
                        ps = pse_pool.tile([128, 2, 512], F32, tag="pe",
                                           name="ps")
                        for k in range(KC):
                            st = enc_ts[q // 2][:, q % 2, tl % 4, k, :]
                            for jh in range(2):
                                nc.tensor.matmul(
                                    ps[:, jh, :], st,
                                    w2r[:, k, jh * 512:(jh + 1) * 512],
                                    start=(k == 0), stop=(k == KC - 1),
                                )
                        if not chain:
                            continue
                        pre = pre_pool.tile([128, H],
                                            BF16 if pre_bf16 else F32,
                                            tag="pre", name="pre")
                        nc.vector.tensor_add(
                            pre[:],
                            ps[:].rearrange("p a b -> p (a b)"),
                            c0_rep[:])
                        th = tanh_pool.tile([128, H], BF16, tag="th",
                                            name="th")
                        nc.scalar.activation(th[:], pre[:], AF.Tanh)
                        prod = prod_pool.tile([128, H], BF16, tag="prod",
                                              name="prod")
                        if use_ttr:
                            nc.vector.tensor_tensor_reduce(
                                out=prod[:], in0=th[:], in1=vrep_sb[:],
                                scale=1.0, scalar=0.0,
                                op0=mybir.AluOpType.mult,
                                op1=mybir.AluOpType.add,
                                accum_out=logits[:, t:t + 1])
                        else:
                            nc.vector.tensor_mul(prod[:], th[:],
                                                  vrep_sb[:])
                            nc.vector.tensor_reduce(
                                logits[:, t:t + 1], prod[:],
                                axis=mybir.AxisListType.X,
                                op=mybir.AluOpType.add)
                # per-iteration: exp over all 32 logit columns
                if chain:
                    nc.scalar.activation(expst[:], logits[:], AF.Exp,
                                         accum_out=sumc[:])
                else:
                    nc.gpsimd.memset(expst[:], 1.0)
                    nc.gpsimd.memset(sumc[:], 1.0)

            c0_section()
            if repeat == 1:
                main_body()
            else:
                with tc.For_i(0, repeat, 1,
                              hint_engines=(mybir.EngineType.PE,)) as _i:
                    main_body(_i)

            # --- softmax normalization across cores -----------------------
            ones_sb = sm_pool.tile([128, 1], F32)
            nc.gpsimd.memset(ones_sb[:], 1.0)
            zp = pse_pool.tile([1, 1], F32, tag="pe", name="zp")
            nc.tensor.matmul(zp[:], ones_sb[:], sumc[:],
                             start=True, stop=True)
            if single_core:
                zg_src = zp
            else:
                ag_in = dram_pool.tile([1, 1], F32)
                zsb = sm_pool.tile([1, 1], F32)
                nc.vector.tensor_copy(zsb[:], zp[:])
                nc.gpsimd.dma_start(ag_in[:], zsb[:])
                ag_out = dram_pool.tile([1, NCORES], F32)
                nc.gpsimd.collective_compute(
                    "AllGather",
                    mybir.AluOpType.bypass,
                    replica_groups=[list(range(NCORES))],
                    ins=[ag_in.opt()],
                    outs=[ag_out.opt()],
                )
                zs = sm_pool.tile([1, NCORES], F32)
                nc.gpsimd.dma_start(zs[:], ag_out[:])
                zg_src = None
            zg = sm_pool.tile([1, 1], F32)
            if single_core:
                nc.vector.tensor_copy(zg[:], zg_src[:])
            else:
                nc.vector.reduce_sum(zg[:], zs[:], axis=mybir.AxisListType.X)
            invz = sm_pool.tile([1, 1], F32)
            nc.vector.reciprocal(invz[:], zg[:])
            invz_rep = sm_pool.tile([128, 1], F32)
            bcast_rows(invz_rep, invz, 1)
            outv = sm_pool.tile([128, NT], F32)
            nc.vector.tensor_scalar_mul(outv[:], expst[:], invz_rep[:])
            nc.sync.dma_start(outT[:], outv[:])

    _dedup_ldweights(nc)
    _split_multi_waits(nc)
    return nc


def build(repeat: int = 1, main_dt: str = "bf16", single_core: bool = False,
          mode: str = "full", exp_sbuf: bool = True, enc_bufs: int = 8,
          tanh_bufs: int = 10, vdot_preload: bool = True,
          vdot_batch: bool = True, dma_rings: bool = False,
          dma_fuse: bool = False, layout: str = "sj", **sj_kw):
    if layout == "sj":
        return build_sj(repeat, main_dt=main_dt, single_core=single_core,
                        enc_bufs=enc_bufs, **sj_kw)
    enc_bufs = min(enc_bufs, 6)   # js SBUF budget (38 tanh bufs)
    """Build the per-core Bass module. `repeat` wraps the main compute in a
    For_i loop (used only by the benchmark harness to measure HW time by
    marginal wall-clock; the softmax tail + collective stay outside).
    mode: full | mm_only (perf experiment: main matmuls + dma only) |
    mm_tanh (mains + dma + tanh, no vdots/exps) | mm_resident (main
    matmuls only, enc preloaded to SBUF outside the loop)."""
    mm_only = mode in ("mm_only", "mm_resident", "mm_halfdma")
    mm_resident = mode == "mm_resident"
    half_dma = mode == "mm_halfdma"
    do_tanh = mode in ("full", "mm_tanh")
    do_vdot = mode == "full"
    if vdot_batch:
        # a full half's th tiles (32) stay alive until the burst, plus
        # the next half's first groups in flight
        tanh_bufs = max(tanh_bufs, 38)
    MD = {"f32r": F32R, "bf16": BF16}[main_dt]
    nc = bass.Bass("TRN2", target_bir_lowering=False, debug=False,
                   num_devices=1 if single_core else NCORES)

    # enc shard pre-tiled on host: [g, i, sq, p, (k s)] (shared with the
    # sj layout; js views it back to [p, k, i, s] per group).
    encC = nc.dram_tensor("encC", [4, 2, 4, 128, KC * 128], MD,
                          kind="ExternalInput").ap()
    w2t = nc.dram_tensor("w2t", [H, H], MD, kind="ExternalInput").ap()
    w1t = nc.dram_tensor("w1t", [2 * H // NCORES, H], F32R,
                         kind="ExternalInput").ap()
    hidT = nc.dram_tensor("hidT", [128, 16 // NCORES], F32R,
                          kind="ExternalInput").ap()
    bias = nc.dram_tensor("bias", [1, H], F32, kind="ExternalInput").ap()
    vwc = nc.dram_tensor("vwc", [128, JC], BF16, kind="ExternalInput").ap()
    # declared by both layouts so one in_map serves either build
    nc.dram_tensor("vrep", [128, H], BF16, kind="ExternalInput")
    out = nc.dram_tensor("out", [1, SL], F32, kind="ExternalOutput").ap()

    encC_v = encC.rearrange("g i q p (k s) -> g p k i q s", k=KC)
    w2t_v = w2t.rearrange("(k p) j -> p k j", p=128)     # [128, 8, 1024]
    w1t_v = w1t.rearrange("(k p) j -> p k j", p=128)     # [128, 2, 1024]

    with tile.TileContext(nc) as tc:
        with (
            tc.tile_pool(name="const", bufs=1) as const_pool,
            tc.tile_pool(name="enc", bufs=enc_bufs) as enc_pool,
            tc.tile_pool(name="tanh", bufs=tanh_bufs) as tanh_pool,
            tc.tile_pool(name="sm", bufs=1) as sm_pool,
            tc.tile_pool(name="pse", bufs=7, space="PSUM") as pse_pool,
            tc.tile_pool(name="psa", bufs=1, space="PSUM") as psa_pool,
            tc.tile_pool(name="dram", bufs=1, space="DRAM") as dram_pool,
        ):
            # --- tiny constants -------------------------------------------
            hid_sb = const_pool.tile([128, 16 // NCORES], F32R)
            nc.sync.dma_start(hid_sb[:], hidT[:])
            vw_sb = const_pool.tile([128, JC], BF16)
            nc.sync.dma_start(vw_sb[:], vwc[:])
            b_sb = const_pool.tile([1, H], F32)
            nc.sync.dma_start(b_sb[:], bias[:])

            # --- replicated weights: one tile per j-slab so the group-j
            # matmuls depend only on their own slab's DMA ---------------
            w2_tiles = []
            for j in range(JC):
                w2_j = const_pool.tile([128, KC, 128], MD, name=f"w2_{j}")
                nc.sync.dma_start(w2_j[:], w2t_v[:, :, j * 128:(j + 1) * 128])
                w2_tiles.append(w2_j)

            exps = sm_pool.tile([1, SL], F32)
            sums = sm_pool.tile([1, NSB], F32)

            # --- c0 = hidden @ W1T + attn_b (one row), sharded over cores
            c0_sb = const_pool.tile([128, JC], F32)

            NKC = 16 // NCORES   # local w1 chunks (c0 sharded over cores)

            def c0_section():
                w1_sb = const_pool.tile([128, NKC, H], F32R)
                nc.sync.dma_start(w1_sb[:], w1t_v[:])
                # bias arrives pre-divided by NCORES, so adding it to the
                # local partial and AllReduce-summing reconstructs c0+b
                part_row = const_pool.tile([1, H], F32)
                for half in range(2):
                    psum_c = pse_pool.tile([1, 512], F32, tag="pe",
                                           name="psum_c")
                    for kc in range(NKC):
                        nc.tensor.matmul(
                            psum_c[:],
                            hid_sb[:, kc:kc + 1],
                            w1_sb[:, kc, half * 512:(half + 1) * 512],
                            start=(kc == 0), stop=(kc == NKC - 1),
                        )
                    nc.vector.tensor_add(
                        part_row[:, half * 512:(half + 1) * 512],
                        psum_c[:],
                        b_sb[:, half * 512:(half + 1) * 512])
                ar_in = dram_pool.tile([1, H], F32)
                nc.gpsimd.dma_start(ar_in[:], part_row[:])
                if single_core:
                    ar_out = ar_in
                else:
                    ar_out = dram_pool.tile([1, H], F32)
                    nc.gpsimd.collective_compute(
                        "AllReduce",
                        mybir.AluOpType.add,
                        replica_groups=[list(range(NCORES))],
                        ins=[ar_in.opt()],
                        outs=[ar_out.opt()],
                    )
                nc.sync.dma_start(
                    c0_sb[:],
                    ar_out[:].rearrange("o (j p) -> (o p) j", p=128)
                )

            # --- main pipeline -------------------------------------------
            enc_res = [None]
            if mm_resident:
                enc_res[0] = const_pool.tile([128, 4, KC, 2, SB], MD,
                                             name="enc_res")
                for g in range(4):
                    nc.sync.dma_start(enc_res[0][:, g], encC_v[g])

            def main_body(_iv=None):
                # per j-group: 4 single-bank psum accumulators (the 4
                # s-blocks of the half), all fed k-outer so the 4 matmuls
                # of a (j, k) pair share one weight load. One [128, SB]
                # psum_a bank whose quadrant rows 0/32/64/96 hold the 4
                # s-blocks' logits so the 4 v-dots of a group land on
                # distinct PE column groups and stream concurrently.
                psum_a = [None]
                pending = []               # delayed v-dot emissions
                last_main = [None]         # latest main matmul instruction

                def flush():
                    for emit in pending:
                        emit()
                    pending.clear()

                def make_vdot(j, ths, pa):
                    def emit():
                        if vdot_preload:
                            # preload all 4 col-group weight slots first,
                            # then issue the 4 matmuls back-to-back so
                            # they stream concurrently (no interleaved
                            # LDW can stall the col-group pipeline; the
                            # per-MM auto-LDWs dedup against these).
                            # Pin each preload behind the latest main
                            # matmul so the scheduler cannot hoist it
                            # into an earlier weight-load's live range.
                            for q in range(4):
                                ldw = nc.tensor.ldweights(
                                    vw_sb[:, j:j + 1],
                                    tile_position=(0, 32 * q))
                                # mirror the fused matmul's rounded tile
                                # size so the per-MM auto-LDW dedups
                                # against this preload
                                ldw.ins.tile_size = (128, 32)
                                if last_main[0] is not None:
                                    bass._add_dep_helper(
                                        ldw.ins, last_main[0],
                                        sync=True,
                                        reason="pin vdot preload")
                        for q in range(4):
                            r = 32 * q
                            nc.tensor.matmul(
                                pa[r:r + 1, :],
                                vw_sb[:, j:j + 1], ths[q][:],
                                tile_position=(0, r),
                                start=(j == 0), stop=(j == JC - 1),
                            )
                    return emit

                def copy_logits(pa):
                    # DVE copies the logits bank to SBUF (~0.7us) so the
                    # psa bank frees fast and ACT's exps read SBUF off the
                    # PE-critical path (DVE is otherwise idle in-loop)
                    if not exp_sbuf:
                        return pa
                    lt = sm_pool.tile([128, SB], F32, tag="lt", name="lt",
                                      bufs=2)
                    nc.vector.tensor_copy(lt[:], pa[:])
                    return lt

                def emit_exps(h, lt):
                    for q in range(4):
                        sb = 4 * h + q
                        nc.scalar.activation(
                            exps[:, sb * SB:(sb + 1) * SB],
                            lt[32 * q:32 * q + 1, :], AF.Exp,
                            accum_out=sums[:, sb:sb + 1],
                        )

                prev_pa = None
                for h in range(2):
                    if mm_resident:
                        enc_ts = [enc_res[0][:, 2 * h + pp]
                                  for pp in range(2)]
                    elif dma_fuse:
                        # one 4MB DMA per half covering both s-block pairs
                        enc_t2 = enc_pool.tile([128, 2, KC, 2, SB], MD,
                                               tag="enc", bufs=enc_bufs // 2)
                        nc.sync.dma_start(
                            enc_t2[:],
                            encC.rearrange("g p x -> p g x")[:, 2 * h:2 * h + 2]
                            .rearrange("p g (k i s) -> p g k i s", k=KC, i=2),
                        )
                        enc_ts = [enc_t2[:, pp] for pp in range(2)]
                    else:
                        enc_ts = []
                        for pp in range(2):     # two s-block pairs per half
                            enc_t = enc_pool.tile([128, KC, 2, SB], MD,
                                                  tag="enc")
                            eng = (nc.scalar if (dma_rings and pp == 1)
                                   else nc.sync)
                            if half_dma:   # perf probe: half the bytes
                                eng.dma_start(enc_t[:, :KC // 2],
                                              encC_v[2 * h + pp][:, :KC // 2])
                            else:
                                eng.dma_start(enc_t[:], encC_v[2 * h + pp])
                            enc_ts.append(enc_t)
                    for j in range(JC):
                        pes = [
                            pse_pool.tile([128, SB], F32, tag="pe",
                                          name="pe")
                            for _ in range(4)
                        ]
                        for k in range(KC):
                            w = w2_tiles[j][:, k, :]
                            for q in range(4):
                                mm = nc.tensor.matmul(
                                    pes[q][:], w,
                                    enc_ts[q // 2][:, k, q % 2, :],
                                    start=(k == 0), stop=(k == KC - 1),
                                )
                                last_main[0] = mm.ins
                        if not do_tanh:
                            continue
                        if not vdot_batch or j == 0:
                            flush()
                        lt_prev = None
                        if do_vdot and j == 0:
                            # previous half's logits complete: DVE-copy
                            # them out before this half's first v-dots
                            # reuse the bank
                            if h == 1:
                                lt_prev = copy_logits(prev_pa)
                            psum_a[0] = psa_pool.tile(
                                [128, SB], F32, tag="psa", name="psa")
                        ths = []
                        for q in range(4):
                            th = tanh_pool.tile([128, SB], BF16,
                                                tag="th", name="th")
                            nc.scalar.activation(
                                th[:], pes[q][:], AF.Tanh,
                                bias=c0_sb[:, j:j + 1])
                            ths.append(th)
                        if lt_prev is not None:
                            # exps queue on ACT after this j's tanhs so
                            # they never delay the psum-bank recycle
                            emit_exps(0, lt_prev)
                        if do_vdot:
                            pending.append(make_vdot(j, ths, psum_a[0]))
                    prev_pa = psum_a[0]
                if do_vdot:
                    flush()
                    emit_exps(1, copy_logits(prev_pa))
                else:
                    nc.gpsimd.memset(exps[:], 1.0)
                    nc.gpsimd.memset(sums[:], 1.0)

            c0_section()
            if repeat == 1:
                main_body()
            else:
                with tc.For_i(0, repeat, 1,
                              hint_engines=(mybir.EngineType.PE,)) as _i:
                    main_body(_i)

            # --- softmax normalization across cores -----------------------
            if single_core:
                zg = sm_pool.tile([1, 1], F32)
                nc.vector.reduce_sum(zg[:], sums[:],
                                     axis=mybir.AxisListType.X)
            else:
                # AllGather the raw per-block sums (8 floats/core) and do a
                # single 64-element reduce afterwards
                ag_in = dram_pool.tile([1, NSB], F32)
                nc.gpsimd.dma_start(ag_in[:], sums[:])
                ag_out = dram_pool.tile([1, NCORES * NSB], F32)
                nc.gpsimd.collective_compute(
                    "AllGather",
                    mybir.AluOpType.bypass,
                    replica_groups=[list(range(NCORES))],
                    ins=[ag_in.opt()],
                    outs=[ag_out.opt()],
                )
                zs = sm_pool.tile([1, NCORES * NSB], F32)
                nc.gpsimd.dma_start(zs[:], ag_out[:])
                zg = sm_pool.tile([1, 1], F32)
                nc.vector.reduce_sum(zg[:], zs[:], axis=mybir.AxisListType.X)
            invz = sm_pool.tile([1, 1], F32)
            nc.vector.reciprocal(invz[:], zg[:])
            outv = sm_pool.tile([1, SL], F32)
            # split the 4096-element scale across ACT and DVE in parallel,
            # and ship each half as soon as it's done
            hl = SL // 2
            nc.scalar.activation(outv[:, :hl], exps[:, :hl], AF.Identity,
                                 scale=invz[:])
            nc.sync.dma_start(out[:, :hl], outv[:, :hl])
            nc.vector.tensor_scalar_mul(outv[:, hl:], exps[:, hl:], invz[:])
            nc.sync.dma_start(out[:, hl:], outv[:, hl:])

    _dedup_ldweights(nc)
    _split_multi_waits(nc)
    return nc


def prepare_in_maps(hidden, encoder_output, attn_w, attn_b, v_w,
                    main_dt="bf16"):
    hidden = np.asarray(hidden, dtype=np.float32)
    enc = np.asarray(encoder_output, dtype=np.float32)
    attn_w = np.asarray(attn_w, dtype=np.float32)
    attn_b = np.asarray(attn_b, dtype=np.float32)
    v_w = np.asarray(v_w, dtype=np.float32)

    import ml_dtypes
    md = np.float32 if main_dt == "f32r" else ml_dtypes.bfloat16
    w2t = np.ascontiguousarray(attn_w[:, 2 * H:].T).astype(md)   # [H, H]
    w1t_full = np.ascontiguousarray(attn_w[:, :2 * H].T)
    hidT_full = np.ascontiguousarray(hidden.reshape(16, 128).T)
    kpc = 16 // NCORES
    b = np.ascontiguousarray(attn_b.reshape(1, H)) / np.float32(NCORES)
    vwc = np.ascontiguousarray(v_w.reshape(JC, 128).T).astype(
        ml_dtypes.bfloat16)  # [128, 8]
    vrep = np.ascontiguousarray(
        np.broadcast_to(v_w.reshape(1, H), (128, H))).astype(
        ml_dtypes.bfloat16)  # [128, H] replicated

    in_maps = []
    for c in range(NCORES):
        encT = enc[c * SL:(c + 1) * SL, :].T.astype(md)   # [H, SL]
        # [g, i, sq, p, k, s128]: every (g, i, sq) quarter contiguous
        # per partition so each quarter-tile DMA is 128 descriptors of 2KB
        encC = np.ascontiguousarray(
            encT.reshape(KC, 128, 4, 2, 4, 128).transpose(2, 3, 4, 1, 0, 5)
        ).reshape(4, 2, 4, 128, KC * 128)
        in_maps.append({
            "encC": encC, "w2t": w2t,
            "w1t": np.ascontiguousarray(
                w1t_full[c * kpc * 128:(c + 1) * kpc * 128, :]),
            "hidT": np.ascontiguousarray(
                hidT_full[:, c * kpc:(c + 1) * kpc]),
            "bias": b, "vwc": vwc, "vrep": vrep,
        })
    return in_maps


_NC_CACHE = {}


def _get_nc(repeat: int = 1):
    if repeat not in _NC_CACHE:
        _NC_CACHE[repeat] = build(repeat)
    return _NC_CACHE[repeat]


def kernel(hidden, encoder_output, attn_w, attn_b, v_w):
    nc = _get_nc(1)
    in_maps = prepare_in_maps(hidden, encoder_output, attn_w, attn_b, v_w)
    res = run_bass_kernel_spmd(nc, in_maps, list(range(NCORES)))
    parts = []
    for c in range(NCORES):
        r = res.results[c]
        if "outT" in r:
            # outT[p, t] holds s_local = t*128 + p
            parts.append(np.ascontiguousarray(r["outT"].T).reshape(SL))
        else:
            parts.append(r["out"][0])
    return np.concatenate(parts)
